# revision 1
# baseline (speedup 1.0000x reference)
"""Sliding-window multi-head attention for Trainium2, 8-core SPMD.

Sharding: sequence-parallel. B=2 batches x 4 chunks of 512 queries = 8 cores.
Each core computes QKV projections for its chunk (+128-row halo for K/V),
banded attention (window 256 -> band |j-s|<=128), and the output projection
for its 512 rows. No collectives; host concatenates the 8 output chunks.

Math notes (validated against the reference):
 - The reference's clamped scatter-add with zero-padded keys is exactly a
   banded score matrix: full[s,j] = q_s.k_j / 8 for |j-s|<=128, -inf outside.
 - Softmax computed without max-subtraction (scores are O(1), no overflow).
 - Denominators come free from the AV matmul via a ones-column on V (M=65).
 - Attention is computed transposed (scores^T[key, query]) so no transposes
   are needed anywhere in the hot loop; q^T/k^T come straight out of the
   projection, V is projected in natural layout for the AV lhsT.
"""

import numpy as np

import concourse.bass as bass
import concourse.tile as tile
from concourse import mybir
from concourse.alu_op_type import AluOpType
from concourse.vector_clock import ScopedClock
from concourse.bass_utils import run_bass_kernel_spmd

FP32 = mybir.dt.float32
FP32R = mybir.dt.float32r


# fp32r (single-pass PE matmul, 2x faster than fp32; ~1.6e-4 relative
# rounding, measured on HW) is threaded through tile dtypes natively: the
# BIR verifier requires every producer of an fp32r-matmul input to round.

# Problem constants (hardcoded per contract)
B, S, IN_DIM, E = 2, 2048, 512, 512
H, HD = 8, 64
WS, HW = 256, 128
CH = 512          # own queries per core
LK = 768          # local keys per core (chunk + 128 halo each side)
NT = 6            # key tiles of 128
W_T = [128, 256, 384, 384, 256, 128]   # valid query-span width per key tile
QS_T = [0, 0, 0, 128, 256, 384]        # local query start per key tile
OFF_T = [0, 128, 384, 768, 1152, 1408]  # column offset in the concat layout
WSUM = 1536

_MAX_WAITS = 1
_patched = False


def _split_sync_waits(nc):
    """This container's walrus accepts only 1 sync-wait per instruction.
    Move extra waits onto nofuse NOPs inserted just before, on the same
    engine sequencer (in-order execution makes this equivalent)."""
    n_split = 0
    for fn in nc.m.functions:
        for bb in fn.blocks:
            insts = list(bb.instructions)
            out = []
            for inst in insts:
                si = inst.sync_info
                if si is not None and len(si.on_wait) > _MAX_WAITS:
                    waits = list(si.on_wait)
                    extra, keep = waits[:-_MAX_WAITS], waits[-_MAX_WAITS:]
                    for j in range(0, len(extra), _MAX_WAITS):
                        out.append(
                            mybir.InstNoOp(
                                name=f"{inst.name}-sw{j}",
                                engine=inst.engine,
                                bass_nofuse=True,
                                sync_info=mybir.SyncInfo(
                                    on_wait=extra[j : j + _MAX_WAITS], on_update=[]
                                ),
                            )
                        )
                    inst.sync_info = mybir.SyncInfo(
                        on_wait=keep, on_update=list(si.on_update)
                    )
                    n_split += 1
                out.append(inst)
            if len(out) != len(insts):
                try:
                    bb.instructions = out
                except Exception:
                    bb.instructions[:] = out
    return n_split


def _patch_tile_drain():
    global _patched
    if _patched:
        return
    _patched = True

    def _drain_and_barrier(self, tick_clock, wait_clock):
        nc = self.nc
        drain_inst = nc.sync.drain()
        wait_clock.add_sem_waits(
            drain_inst.ins, ScopedClock({None: tick_clock.global_clock})
        )
        nc.all_engine_barrier()
        assert self.sems is not None
        popped = nc._tile_sem_poison_stack.pop()
        assert popped is self._sem_poison
        nc.clear_and_free_semaphores(list(self.sems.allocated().values()))
        nc.all_engine_barrier()
        _split_sync_waits(nc)

    tile.TileContext._drain_and_barrier = _drain_and_barrier


def _build_program():
    _patch_tile_drain()
    nc = bass.Bass("TRN2", target_bir_lowering=False, debug=False)

    xt = nc.dram_tensor("xt", [IN_DIM, LK], FP32R, kind="ExternalInput")
    wq = nc.dram_tensor("wq", [IN_DIM, E], FP32R, kind="ExternalInput")
    wk = nc.dram_tensor("wk", [IN_DIM, E], FP32R, kind="ExternalInput")
    wv = nc.dram_tensor("wv", [IN_DIM, E], FP32R, kind="ExternalInput")
    ow = nc.dram_tensor("ow", [E, E], FP32R, kind="ExternalInput")
    qb = nc.dram_tensor("qb", [4, 128], FP32, kind="ExternalInput")
    kb = nc.dram_tensor("kb", [4, 128], FP32, kind="ExternalInput")
    vb = nc.dram_tensor("vb", [128, E], FP32, kind="ExternalInput")
    ob = nc.dram_tensor("ob", [128, E], FP32, kind="ExternalInput")
    mk = nc.dram_tensor("mk", [128, WSUM], FP32R, kind="ExternalInput")
    out = nc.dram_tensor("out", [CH, E], FP32, kind="ExternalOutput")

    with tile.TileContext(nc) as tc:
        with (
            tc.tile_pool(name="const", bufs=1) as cpool,
            tc.tile_pool(name="proj", bufs=1) as ppool,
            tc.tile_pool(name="att", bufs=3) as apool,
            tc.tile_pool(name="small", bufs=2) as spool,
            tc.tile_pool(name="fin", bufs=2) as fpool,
            tc.tile_pool(name="ps2", bufs=2, space="PSUM") as ps2,
            tc.tile_pool(name="ps3", bufs=3, space="PSUM") as ps3,
            tc.tile_pool(name="ps1", bufs=1, space="PSUM") as ps1,
        ):
            # ---- loads, ordered so the q/k projections can start ASAP ----
            def load(pool_tag, shape, dt, ap):
                t = cpool.tile(shape, dt, tag=pool_tag)
                nc.sync.dma_start(t[:], ap)
                return t

            # x + weights; alternate the two HWDGE queues (sync=SP,
            # scalar=ACT) so the ~0.6us per-DMA dispatch doesn't serialize
            # the front of the kernel.
            def load(pool_tag, shape, dt, ap, eng):
                t = cpool.tile(shape, dt, tag=pool_tag, name=pool_tag)
                eng.dma_start(t[:], ap)
                return t

            xt_t = [load(f"xt{p}", [128, LK], FP32R, xt[128 * p : 128 * p + 128, :],
                         nc.sync if p % 2 == 0 else nc.scalar) for p in range(4)]
            wq_t = [load(f"wq{p}", [128, E], FP32R, wq[128 * p : 128 * p + 128, :],
                         nc.sync if p % 2 == 0 else nc.scalar) for p in range(4)]
            qb_t = [load(f"qb{p}", [128, 1], FP32, qb[p, :][:, None], nc.sync) for p in range(4)]
            kb_t = [load(f"kb{p}", [128, 1], FP32, kb[p, :][:, None], nc.scalar) for p in range(4)]
            wk_t = [load(f"wk{p}", [128, E], FP32R, wk[128 * p : 128 * p + 128, :],
                         nc.sync if p % 2 == 0 else nc.scalar) for p in range(4)]
            wv_t = [load(f"wv{p}", [128, E], FP32R, wv[128 * p : 128 * p + 128, :],
                         nc.sync if p % 2 == 0 else nc.scalar) for p in range(4)]
            vb_t = load("vb", [128, E], FP32, vb[:], nc.scalar)
            mk_t = load("mk", [128, WSUM], FP32R, mk[:], nc.sync)
            ow_t = [load(f"ow{p}", [128, E], FP32R, ow[128 * p : 128 * p + 128, :],
                         nc.sync if p % 2 == 0 else nc.scalar) for p in range(4)]
            ob_t = load("ob", [128, E], FP32, ob[:], nc.scalar)
            ones_t = cpool.tile([1, 64], FP32, tag="ones")
            nc.vector.memset(ones_t[:], 1.0)

            # HAM warmup: dummy matmuls with no DMA deps run while the
            # input DMAs stream in, so the PE clock gate is already at
            # 8/8 when the first projection matmul issues. They cycle
            # through the psO slots, which have no real user until AV(0).
            dum_t = cpool.tile([128, E], FP32R, tag="dum")
            nc.vector.memset(dum_t[:].bitcast(FP32), 0.0)
            for i in range(24):
                psd = ps2.tile([HD + 1, CH], FP32, name=f"dum{i}", tag="psO")
                nc.tensor.matmul(
                    psd[:, 0:CH], dum_t[:, 0:HD + 1], dum_t[:, 0:CH],
                    start=True, stop=True,
                )

            qT = [None] * 4
            kT = [None] * 4

            # q/k projection for pair p, split into 3 chunks so it can be
            # interleaved into the previous pair's attention (keeps the PE
            # dense with N=512 matmuls so HAM stays at full clock)
            def emit_qk_chunk(p, chunk):
                if chunk == 0:
                    psq = ps2.tile([128, CH], FP32, tag="ps_big")
                    for kk in range(4):
                        nc.tensor.matmul(
                            psq[:],
                            wq_t[kk][:, 128 * p : 128 * p + 128],
                            xt_t[kk][:, 128 : 128 + CH],
                            start=(kk == 0), stop=(kk == 3),
                        )
                    q = ppool.tile([128, CH], FP32R, tag=f"qT{p}")
                    nc.vector.tensor_scalar_add(q[:], psq[:], qb_t[p][:, 0:1])
                    qT[p] = q
                else:
                    h = chunk - 1
                    if h == 0:
                        kT[p] = ppool.tile([128, LK], FP32R, name=f"kT{p}", tag=f"kT{p}")
                    psk = ps3.tile([128, 384], FP32, tag="ps_s")
                    for kk in range(4):
                        nc.tensor.matmul(
                            psk[:],
                            wk_t[kk][:, 128 * p : 128 * p + 128],
                            xt_t[kk][:, 384 * h : 384 * h + 384],
                            start=(kk == 0), stop=(kk == 3),
                        )
                    nc.vector.tensor_scalar_add(
                        kT[p][:, 384 * h : 384 * h + 384], psk[:], kb_t[p][:, 0:1]
                    )

            for p in range(4):
                for c in range(3):
                    emit_qk_chunk(p, c)

            # v in natural layout [keys, 8*(64+1)]: per head 64 v-cols + ones
            v_t = []
            for m in range(NT):
                psv = ps2.tile([128, E], FP32, tag="ps_big")
                for kk in range(4):
                    nc.tensor.matmul(
                        psv[:],
                        xt_t[kk][:, 128 * m : 128 * m + 128],
                        wv_t[kk][:],
                        start=(kk == 0), stop=(kk == 3),
                    )
                v = ppool.tile([128, H * (HD + 1)], FP32R, tag=f"v{m}")
                v3 = v[:].rearrange("p (h d) -> p h d", d=HD + 1)
                psv3 = psv[:].rearrange("p (h d) -> p h d", d=HD)
                vb3 = vb_t[:].rearrange("p (h d) -> p h d", d=HD)
                nc.vector.tensor_tensor(v3[:, :, 0:HD], psv3, vb3, op=AluOpType.add)
                nc.vector.memset(v3[:, :, HD : HD + 1].bitcast(FP32), 1.0)
                v_t.append(v)

            # ---- attention (per pair of heads sharing a 128-row tile) ----
            # scores^T via row-packed K=64 QK pairs, exp on ACT, band mask
            # split across DVE (head A) / GpSimd (head B). Emission is
            # pipeline-shifted: QK phase of pair p+1 goes BEFORE the AV
            # phase of pair p, so the PE has independent matmuls to run
            # while ACT/DVE/GpSimd chew through pair p's exps and masks.
            att_tiles = {}
            pso_tiles = {}

            def emit_av_tile(p, hh, t):
                att = att_tiles[p][hh]
                pso = pso_tiles[p][hh]
                head = 2 * p + hh
                w, qs, off = W_T[t], QS_T[t], OFF_T[t]
                nc.tensor.matmul(
                    pso[:, qs : qs + w],
                    v_t[t][:, (HD + 1) * head : (HD + 1) * head + HD + 1],
                    att[:, off : off + w],
                    start=(t == 0), stop=(t == NT - 1),
                    skip_group_check=True,
                )

            def emit_qk_phase(p, av_pair=None):
                attA = apool.tile([128, WSUM], FP32R, name=f"attA{p}", tag="attA")
                attB = apool.tile([128, WSUM], FP32R, name=f"attB{p}", tag="attB")
                att_tiles[p] = (attA, attB)
                if av_pair is not None:
                    pso_tiles[av_pair] = (
                        ps2.tile([HD + 1, CH], FP32, name=f"pso{2 * av_pair}", tag="psO"),
                        ps2.tile([HD + 1, CH], FP32, name=f"pso{2 * av_pair + 1}", tag="psO"),
                    )
                for t in range(NT):
                    w, qs, off = W_T[t], QS_T[t], OFF_T[t]
                    pa = ps3.tile([128, 384], FP32, name=f"pa{p}_{t}", tag="ps_s")
                    nc.tensor.matmul(
                        pa[:, 0:w],
                        kT[p][0:64, 128 * t : 128 * t + 128],
                        qT[p][0:64, qs : qs + w],
                        start=True, stop=True,
                    )
                    pb = ps2.tile([128, 512], FP32, name=f"pb{p}_{t}", tag="ps_big")
                    nc.tensor.matmul(
                        pb[:, 0:w],
                        kT[p][64:128, 128 * t : 128 * t + 128],
                        qT[p][64:128, qs : qs + w],
                        start=True, stop=True,
                    )
                    # previous pair's AV matmuls fill the PE stream here
                    # while ACT/DVE/GpSimd run this pair's exps and masks
                    if av_pair is not None:
                        emit_av_tile(av_pair, 0, t)
                        emit_av_tile(av_pair, 1, t)
                    nc.scalar.activation(
                        attA[:, off : off + w], pa[:, 0:w],
                        mybir.ActivationFunctionType.Exp,
                    )
                    nc.scalar.activation(
                        attB[:, off : off + w], pb[:, 0:w],
                        mybir.ActivationFunctionType.Exp,
                    )
                    nc.vector.tensor_mul(
                        attA[:, off : off + w], attA[:, off : off + w],
                        mk_t[:, off : off + w],
                    )
                    nc.gpsimd.tensor_mul(
                        attB[:, off : off + w], attB[:, off : off + w],
                        mk_t[:, off : off + w],
                    )

            def emit_av_phase(p, tiles):
                # AV: psO[0:64] = V^T @ att^T (unnormalized values^T),
                #     psO[64]   = column sums (softmax denominators).
                if tiles:
                    pso_tiles[p] = (
                        ps2.tile([HD + 1, CH], FP32, name=f"pso{2 * p}", tag="psO"),
                        ps2.tile([HD + 1, CH], FP32, name=f"pso{2 * p + 1}", tag="psO"),
                    )
                    for t in range(NT):
                        emit_av_tile(p, 0, t)
                        emit_av_tile(p, 1, t)
                vtn = ppool.tile([128, CH], FP32R, name=f"vT{p}", tag=f"vT{p}")
                rbc = ps1.tile([128, CH], FP32, name=f"rbc{p}", tag="rbc")
                psos = pso_tiles[p]
                for hh in range(2):
                    head = 2 * p + hh
                    pso = psos[hh]
                    # denominator row -> SBUF, broadcast across 64
                    # partitions with a K=1 matmul into the pair's rbc bank
                    den = spool.tile([1, CH], FP32, name=f"den{head}", tag="den")
                    nc.vector.tensor_copy(den[:], pso[HD : HD + 1, :])
                    nc.tensor.matmul(
                        rbc[64 * hh : 64 * hh + 64, :], ones_t[:], den[:],
                        start=True, stop=True,
                    )
                # reciprocal of both heads' denominators at once via ACT
                # exp(-ln(x)) (DVE reciprocal is ~3.4us/op; ACT Reciprocal
                # is blocked by bass), then scale values per head.
                lnv = spool.tile([128, CH], FP32, name=f"lnv{p}", tag="lnv")
                nc.scalar.activation(
                    lnv[:], rbc[:], mybir.ActivationFunctionType.Ln,
                )
                rbs = spool.tile([128, CH], FP32, name=f"rbs{p}", tag="rbs")
                nc.scalar.activation(
                    rbs[:], lnv[:],
                    mybir.ActivationFunctionType.Exp, scale=-1.0,
                )
                for hh in range(2):
                    nc.vector.tensor_mul(
                        vtn[64 * hh : 64 * hh + 64, :],
                        psos[hh][0:HD, :], rbs[64 * hh : 64 * hh + 64, :],
                    )
                vT.append(vtn)

            vT = []
            emit_qk_phase(0)
            for p in range(4):
                if p < 3:
                    emit_qk_phase(p + 1)
                emit_av_phase(p, tiles=True)

            # ---- output projection ----
            for m in range(4):
                psf = ps2.tile([128, E], FP32, tag="ps_big")
                for p in range(4):
                    nc.tensor.matmul(
                        psf[:],
                        vT[p][:, 128 * m : 128 * m + 128],
                        ow_t[p][:],
                        start=(p == 0), stop=(p == 3),
                    )
                fin = fpool.tile([128, E], FP32, tag="fin")
                nc.vector.tensor_tensor(fin[:], psf[:], ob_t[:], op=AluOpType.add)
                nc.sync.dma_start(out[128 * m : 128 * m + 128, :], fin[:])

    return nc


_NC_CACHE = None


def _get_program():
    global _NC_CACHE
    if _NC_CACHE is None:
        _NC_CACHE = _build_program()
    return _NC_CACHE


def _make_in_maps(x, padding_mask, qkv_w, qkv_b, o_w, o_b):
    x = np.asarray(x, np.float32)
    pm = np.asarray(padding_mask)
    qkv_w = np.asarray(qkv_w, np.float32)
    qkv_b = np.asarray(qkv_b, np.float32)
    o_w = np.asarray(o_w, np.float32)
    o_b = np.asarray(o_b, np.float32)

    scale = np.float32(1.0 / np.sqrt(HD))
    # reference splits per-head: head h uses qkv rows [192h,192h+64) (q),
    # +64 (k), +128 (v)
    idx_q = np.concatenate([np.arange(3 * HD * h, 3 * HD * h + HD) for h in range(H)])
    idx_k = idx_q + HD
    idx_v = idx_q + 2 * HD

    wq = np.ascontiguousarray((qkv_w[idx_q] * scale).T)      # [IN, E]
    wk = np.ascontiguousarray(qkv_w[idx_k].T)
    wv = np.ascontiguousarray(qkv_w[idx_v].T)
    qb = np.ascontiguousarray((qkv_b[idx_q] * scale).reshape(4, 128))
    kb = np.ascontiguousarray(qkv_b[idx_k].reshape(4, 128))
    vb = np.ascontiguousarray(
        np.broadcast_to(qkv_b[idx_v][None, :], (128, E))
    )
    ow = np.ascontiguousarray(o_w.T)                          # [E_in, E_out]
    ob = np.ascontiguousarray(np.broadcast_to(o_b[None, :], (128, E)))

    j = np.arange(128)[:, None]
    in_maps = []
    for c in range(8):
        b, ch = divmod(c, 4)
        s0 = CH * ch
        lo, hi = max(0, s0 - HW), min(S, s0 + CH + HW)
        xpad = np.zeros((LK, IN_DIM), np.float32)
        xpad[lo - (s0 - HW) : hi - (s0 - HW)] = x[b, lo:hi]
        xt = np.ascontiguousarray(xpad.T)                     # [IN, LK]

        mask = np.zeros((128, WSUM), np.float32)
        for t in range(NT):
            w, qs, off = W_T[t], QS_T[t], OFF_T[t]
            lk = 128 * t + j                                  # [128,1] local key
            q = qs + np.arange(w)[None, :]                    # [1,w] local query
            band = (q <= lk) & (lk <= q + 2 * HW)
            gk = s0 - HW + lk                                 # global key index
            valid = (gk >= 0) & (gk < S)
            pmk = pm[b, np.clip(gk, 0, S - 1)] != 0
            mask[:, off : off + w] = (band & valid & pmk).astype(np.float32)

        in_maps.append(
            {"xt": xt, "wq": wq, "wk": wk, "wv": wv, "ow": ow,
             "qb": qb, "kb": kb, "vb": vb, "ob": ob, "mk": mask}
        )
    return in_maps


def _run(x, padding_mask, qkv_w, qkv_b, o_w, o_b, trace=False, tmpdir=None):
    nc = _get_program()
    in_maps = _make_in_maps(x, padding_mask, qkv_w, qkv_b, o_w, o_b)
    res = run_bass_kernel_spmd(
        nc, in_maps, core_ids=list(range(8)), trace=trace, tmpdir=tmpdir
    )
    o = np.empty((B, S, E), np.float32)
    for c in range(8):
        b, ch = divmod(c, 4)
        o[b, CH * ch : CH * ch + CH, :] = res.results[c]["out"]
    # fully-masked query rows: att = 0 -> output is exactly the bias
    pm = np.asarray(padding_mask)
    if (pm == 0).any():
        o[pm == 0] = np.asarray(o_b, np.float32)
    return o, res


def kernel(x, padding_mask, qkv_w, qkv_b, o_w, o_b, window_size, num_heads):
    assert int(window_size) == WS and int(num_heads) == H
    assert tuple(np.asarray(x).shape) == (B, S, IN_DIM)
    o, _ = _run(x, padding_mask, qkv_w, qkv_b, o_w, o_b)
    return o



# revision 16
# speedup vs baseline: 1.1252x; 1.1252x over previous
"""Sliding-window multi-head attention for Trainium2, 8-core SPMD. v2.

Sharding: sequence-parallel. B=2 batches x 4 chunks of 512 queries = 8 cores.
Each core computes QKV projections for its chunk (+128-row halo for K/V),
banded attention (window 256 -> band |j-s|<=128), and the output projection
for its 512 rows. No collectives; host concatenates the 8 output chunks.

v2 vs v1 (116us): bf16 operands end-to-end (fp32 PSUM accumulation), whole-
pair [128,1536] score tiles spanning 3 PSUM banks so exp is ONE activation
per head (was 6), big concatenated input DMAs (3 instead of ~21), biases
folded into matmul K=1 rows or ACT-evacuation bias APs (kills the 1x-rate
DVE tensor ops), reciprocal via the single custom-DVE op, and an emission
order that keeps the PE warm (HAM throttling cost v1 ~30us).

Math notes (validated against the reference):
 - The reference's clamped scatter-add with zero-padded keys is exactly a
   banded score matrix: full[s,j] = q_s.k_j / 8 for |j-s|<=128, -inf outside.
 - Softmax computed without max-subtraction (scores are O(1), no overflow).
 - Denominators come free from the AV matmul via a ones-column on V (M=65).
 - Attention is computed transposed (scores^T[key, query]) so no transposes
   are needed anywhere in the hot loop.
"""

import numpy as np
import ml_dtypes

import concourse.bass as bass
import concourse.tile as tile
from concourse import mybir
from concourse.alu_op_type import AluOpType
from concourse.vector_clock import ScopedClock
from concourse.bass_utils import run_bass_kernel_spmd

FP32 = mybir.dt.float32
FP32R = mybir.dt.float32r
BF16 = mybir.dt.bfloat16
BFNP = ml_dtypes.bfloat16

# Problem constants (hardcoded per contract)
B, S, IN_DIM, E = 2, 2048, 512, 512
H, HD = 8, 64
WS, HW = 256, 128
CH = 512          # own queries per core
LK = 768          # local keys per core (chunk + 128 halo each side)
NT = 6            # key tiles of 128
W_T = [128, 256, 384, 384, 256, 128]   # valid query-span width per key tile
QS_T = [0, 0, 0, 128, 256, 384]        # local query start per key tile
OFF_T = [0, 128, 384, 768, 1152, 1408]  # column offset in the concat layout
WSUM = 1536
# QK segments split so no matmul output crosses a 512-col PSUM bank boundary:
# (key_tile, query_start, width, concat_offset)
SEGS = [(0, 0, 128, 0), (1, 0, 256, 128), (2, 0, 128, 384), (2, 128, 256, 512),
        (3, 128, 256, 768), (3, 384, 128, 1024), (4, 256, 256, 1152),
        (5, 384, 128, 1408)]

# cat0: xt (4x768) + wq (4x512)    [128, 5120]
# cat1: wk (4x512) + wv_ext(4x520) [128, 4128]
# cat2: ow (4x512) + mk (1536)     [128, 3584]
F0, F1, F2 = 4 * LK + 4 * 512, 4 * 512 + 4 * 520, 4 * 512 + WSUM

_MAX_WAITS = 1
_patched = False


def _split_sync_waits(nc):
    """This container's walrus accepts only 1 sync-wait per instruction.
    Move extra waits onto nofuse NOPs inserted just before, on the same
    engine sequencer (in-order execution makes this equivalent)."""
    n_split = 0
    for fn in nc.m.functions:
        for bb in fn.blocks:
            insts = list(bb.instructions)
            out = []
            for inst in insts:
                si = inst.sync_info
                if si is not None and len(si.on_wait) > _MAX_WAITS:
                    waits = list(si.on_wait)
                    extra, keep = waits[:-_MAX_WAITS], waits[-_MAX_WAITS:]
                    for j in range(0, len(extra), _MAX_WAITS):
                        out.append(
                            mybir.InstNoOp(
                                name=f"{inst.name}-sw{j}",
                                engine=inst.engine,
                                bass_nofuse=True,
                                sync_info=mybir.SyncInfo(
                                    on_wait=extra[j : j + _MAX_WAITS], on_update=[]
                                ),
                            )
                        )
                    inst.sync_info = mybir.SyncInfo(
                        on_wait=keep, on_update=list(si.on_update)
                    )
                    n_split += 1
                out.append(inst)
            if len(out) != len(insts):
                try:
                    bb.instructions = out
                except Exception:
                    bb.instructions[:] = out
    return n_split


def _patch_tile_drain():
    global _patched
    if _patched:
        return
    _patched = True

    def _drain_and_barrier(self, tick_clock, wait_clock):
        nc = self.nc
        drain_inst = nc.sync.drain()
        wait_clock.add_sem_waits(
            drain_inst.ins, ScopedClock({None: tick_clock.global_clock})
        )
        nc.all_engine_barrier()
        assert self.sems is not None
        popped = nc._tile_sem_poison_stack.pop()
        assert popped is self._sem_poison
        nc.clear_and_free_semaphores(list(self.sems.allocated().values()))
        nc.all_engine_barrier()
        _split_sync_waits(nc)

    tile.TileContext._drain_and_barrier = _drain_and_barrier


def _build_program():
    _patch_tile_drain()
    nc = bass.Bass("TRN2", target_bir_lowering=False, debug=False)

    cat0 = nc.dram_tensor("cat0", [128, F0], BF16, kind="ExternalInput")
    cat1 = nc.dram_tensor("cat1", [128, F1], BF16, kind="ExternalInput")
    cat2 = nc.dram_tensor("cat2", [128, F2], BF16, kind="ExternalInput")
    qkb = nc.dram_tensor("qkb", [128, 8], FP32, kind="ExternalInput")
    rows2 = nc.dram_tensor("rows2", [1, 1032], BF16, kind="ExternalInput")
    out = nc.dram_tensor("out", [CH, E], FP32, kind="ExternalOutput")

    Exp = mybir.ActivationFunctionType.Exp
    Ln = mybir.ActivationFunctionType.Ln
    Ident = mybir.ActivationFunctionType.Identity
    Copy = mybir.ActivationFunctionType.Copy

    with tile.TileContext(nc) as tc:
        with (
            tc.tile_pool(name="const", bufs=1) as cpool,
            tc.tile_pool(name="proj", bufs=1) as ppool,
            tc.tile_pool(name="att", bufs=2) as apool,
            tc.tile_pool(name="small", bufs=2) as spool,
            tc.tile_pool(name="ps", bufs=2, space="PSUM") as ps,
        ):
            # ---- input DMAs: 3 big concats + 2 small, split over both
            # HWDGE queues (sync=SP, scalar=ACT) ----
            cat0_t = cpool.tile([128, F0], BF16, tag="cat0", name="cat0_t")
            cat1_t = cpool.tile([128, F1], BF16, tag="cat1", name="cat1_t")
            cat2_t = cpool.tile([128, F2], BF16, tag="cat2", name="cat2_t")
            qkb_t = cpool.tile([128, 8], FP32, tag="qkb", name="qkb_t")
            rows2_t = cpool.tile([1, 1032], BF16, tag="rows2", name="rows2_t")
            nc.scalar.dma_start(qkb_t[:], qkb[:])
            nc.scalar.dma_start(rows2_t[:], rows2[:])
            nc.sync.dma_start(cat0_t[:], cat0[:])
            nc.scalar.dma_start(cat1_t[:], cat1[:])
            nc.sync.dma_start(cat2_t[:], cat2[:])

            xt_t = [cat0_t[:, LK * k : LK * k + LK] for k in range(4)]
            wq_t = [cat0_t[:, 4 * LK + 512 * k :][:, :512] for k in range(4)]
            wk_t = [cat1_t[:, 512 * k : 512 * k + 512] for k in range(4)]
            wv_t = [cat1_t[:, 2048 + 520 * k :][:, :520] for k in range(4)]
            ow_t = [cat2_t[:, 512 * k : 512 * k + 512] for k in range(4)]
            mk_t = cat2_t[:, 2048 : 2048 + WSUM]
            vb_row = rows2_t[0:1, 0:520]
            ob_row = rows2_t[0:1, 520:1032]

            # on-chip constants (no DMA deps)
            ones1 = cpool.tile([1, 128], BF16, tag="ones1", name="ones1")
            nc.vector.memset(ones1[:], 1.0)
            ones64 = cpool.tile([1, 64], FP32, tag="ones64", name="ones64")
            nc.vector.memset(ones64[:], 1.0)
            dum = cpool.tile([128, 512], BF16, tag="dum", name="dum")
            nc.vector.memset(dum[:], 0.0)

            # HAM warmup: dummy matmuls with no DMA deps run while the input
            # DMAs stream in, so the PE clock gate is at 8/8 when the first
            # projection matmul issues. 'sc' tag slots have no user until QK0.
            for i in range(14):
                psd = ps.tile([128, 1536], FP32, tag="sc", name=f"dum{i}")
                nc.tensor.matmul(
                    psd[:, 0:512], dum[:, 0:128], dum[:, 0:512],
                    start=True, stop=True,
                )

            # ---- projections ----
            # qT[p]: [128 ch(2 heads), 512 q]; kT[p]: [128 ch, 768 keys]
            # biases applied on ACT during evacuation (per-partition bias AP)
            qT, kT = [None] * 4, [None] * 4
            for p in range(4):
                psq = ps.tile([128, 512], FP32, tag="ps1", name=f"psq{p}")
                for kk in range(4):
                    nc.tensor.matmul(
                        psq[:],
                        wq_t[kk][:, 128 * p : 128 * p + 128],
                        xt_t[kk][:, 128 : 128 + CH],
                        start=(kk == 0), stop=(kk == 3),
                    )
                q = ppool.tile([128, CH], BF16, tag=f"qT{p}", name=f"qT{p}")
                nc.scalar.activation(q[:], psq[:], Ident, bias=qkb_t[:, p : p + 1])
                qT[p] = q
                k_ = ppool.tile([128, LK], BF16, tag=f"kT{p}", name=f"kT{p}")
                kT[p] = k_
                for h, (c0, cw) in enumerate([(0, 512), (512, 256)]):
                    psk = ps.tile([128, cw], FP32, tag="ps1", name=f"psk{p}_{h}")
                    for kk in range(4):
                        nc.tensor.matmul(
                            psk[:],
                            wk_t[kk][:, 128 * p : 128 * p + 128],
                            xt_t[kk][:, c0 : c0 + cw],
                            start=(kk == 0), stop=(kk == 3),
                        )
                    nc.scalar.activation(
                        k_[:, c0 : c0 + cw], psk[:], Ident,
                        bias=qkb_t[:, 4 + p : 5 + p],
                    )

            # v in natural layout [keys, 8*(64+1)]: per head 64 v-cols + a
            # ones column (built by the K=1 bias-row matmul: wv_ext has zero
            # weight cols there and vb_row has 1.0) -> denominators free.
            v_t = []

            def emit_v(m):
                v = ppool.tile([128, 520], BF16, tag=f"v{m}", name=f"v{m}")
                for hh in range(2):
                    c0 = 260 * hh
                    psv = ps.tile([128, 260], FP32, tag="ps1", name=f"psv{m}_{hh}")
                    for kk in range(4):
                        nc.tensor.matmul(
                            psv[:],
                            xt_t[kk][:, 128 * m : 128 * m + 128],
                            wv_t[kk][:, c0 : c0 + 260],
                            start=(kk == 0), stop=False,
                        )
                    nc.tensor.matmul(
                        psv[:], ones1[:], vb_row[:, c0 : c0 + 260],
                        start=False, stop=True,
                    )
                    nc.vector.tensor_copy(v[:, c0 : c0 + 260], psv[:])
                v_t.append(v)

            for m in range(3):
                emit_v(m)

            # ---- attention ----
            att_tiles = {}
            pso_tiles = {}
            den_tiles = {}
            rbs_tiles = {}
            vT = [None] * 4

            def emit_qk(p):
                # row-packed pairs: head A on PE rows 0-63, head B on 64-127,
                # concurrent via tile_position auto-derive from base_partition
                scA = ps.tile([128, WSUM], FP32, tag="sc", name=f"scA{p}")
                scB = ps.tile([128, WSUM], FP32, tag="sc", name=f"scB{p}")
                att_tiles[p] = (
                    apool.tile([128, WSUM], BF16, tag="attA", name=f"attA{p}"),
                    apool.tile([128, WSUM], BF16, tag="attB", name=f"attB{p}"),
                )
                for (t, qs, w, off) in SEGS:
                    nc.tensor.matmul(
                        scA[:, off : off + w],
                        kT[p][0:64, 128 * t : 128 * t + 128],
                        qT[p][0:64, qs : qs + w],
                        start=True, stop=True,
                    )
                    nc.tensor.matmul(
                        scB[:, off : off + w],
                        kT[p][64:128, 128 * t : 128 * t + 128],
                        qT[p][64:128, qs : qs + w],
                        start=True, stop=True,
                    )
                return scA, scB

            def emit_exp_masks(p, scA, scB):
                attA, attB = att_tiles[p]
                nc.scalar.activation(attA[:], scA[:], Exp)
                nc.scalar.activation(attB[:], scB[:], Exp)
                # A-mask on GpSimd (ready first, AV emits A-half first),
                # B-mask on DVE (faster engine for the later-ready tile)
                nc.gpsimd.tensor_tensor(attA[:], attA[:], mk_t, op=AluOpType.mult)
                nc.vector.tensor_tensor(attB[:], attB[:], mk_t, op=AluOpType.mult)

            def emit_av(p):
                attA, attB = att_tiles[p]
                psoA = ps.tile([HD + 1, CH], FP32, tag="ps1", name=f"psoA{p}")
                psoB = ps.tile([HD + 1, CH], FP32, tag="ps1", name=f"psoB{p}")
                pso_tiles[p] = (psoA, psoB)
                for hh, (pso, att) in enumerate([(psoA, attA), (psoB, attB)]):
                    head = 2 * p + hh
                    for t in range(NT):
                        w, qs, off = W_T[t], QS_T[t], OFF_T[t]
                        nc.tensor.matmul(
                            pso[:, qs : qs + w],
                            v_t[t][:, 65 * head : 65 * head + 65],
                            att[:, off : off + w],
                            start=(t == 0), stop=(t == NT - 1),
                            skip_group_check=True,
                        )
                # denominator rows -> SBUF (DVE), off ACT's critical path
                denA = spool.tile([1, CH], FP32, tag="denA", name=f"denA{p}")
                denB = spool.tile([1, CH], FP32, tag="denB", name=f"denB{p}")
                nc.vector.tensor_copy(denA[:], psoA[HD : HD + 1, :])
                nc.vector.tensor_copy(denB[:], psoB[HD : HD + 1, :])
                return denA, denB

            def emit_rbc(p, den):
                # broadcast den rows to 64 partitions each via K=1 matmuls,
                # then 1/x = exp(-ln x) on ACT (container lacks the custom
                # DVE recip op; DVE iterative reciprocal is ~8 cyc/elem)
                denA, denB = den
                rbc = ps.tile([128, CH], FP32, tag="sc", name=f"rbc{p}")
                nc.tensor.matmul(
                    rbc[0:64, :], ones64[:], denA[:],
                    start=True, stop=True,
                )
                nc.tensor.matmul(
                    rbc[64:128, :], ones64[:], denB[:],
                    start=True, stop=True, skip_group_check=True,
                )
                lnv = spool.tile([128, CH], FP32, tag="lnv", name=f"lnv{p}")
                nc.scalar.activation(lnv[:], rbc[:], Ln)
                rbs = spool.tile([128, CH], FP32, tag="rbs", name=f"rbs{p}")
                rbs_tiles[p] = rbs
                nc.scalar.activation(rbs[:], lnv[:], Exp, scale=-1.0)

            def emit_vtn(p):
                psoA, psoB = pso_tiles[p]
                rbs = rbs_tiles[p]
                vtn = ppool.tile([128, CH], BF16, tag=f"vT{p}", name=f"vT{p}")
                nc.vector.tensor_tensor(
                    vtn[0:64, :], psoA[0:HD, :], rbs[0:64, :], op=AluOpType.mult
                )
                nc.vector.tensor_tensor(
                    vtn[64:128, :], psoB[0:HD, :], rbs[64:128, :], op=AluOpType.mult
                )
                vT[p] = vtn

            # schedule: QK0, V3-5 (covers exp0 latency), QK1, AV0, QK2,
            # AV1+rbc0+vtn0, QK3, AV2+rbc1+vtn1, AV3+rbc2+vtn2, rbc3+vtn3
            sc0 = emit_qk(0)
            emit_exp_masks(0, *sc0)
            for m in range(3, 6):
                emit_v(m)
            sc1 = emit_qk(1)
            emit_exp_masks(1, *sc1)
            rec0 = emit_av(0)
            sc2 = emit_qk(2)
            emit_exp_masks(2, *sc2)
            rec1 = emit_av(1)
            emit_rbc(0, rec0)
            emit_vtn(0)
            sc3 = emit_qk(3)
            emit_exp_masks(3, *sc3)
            rec2 = emit_av(2)
            emit_rbc(1, rec1)
            emit_vtn(1)
            rec3 = emit_av(3)
            emit_rbc(2, rec2)
            emit_vtn(2)
            emit_rbc(3, rec3)
            emit_vtn(3)

            # ---- output projection (ob via K=1 bias row) ----
            for m in range(4):
                psf = ps.tile([128, E], FP32, tag="sc", name=f"psf{m}")
                for p in range(4):
                    nc.tensor.matmul(
                        psf[:, 0:E],
                        vT[p][:, 128 * m : 128 * m + 128],
                        ow_t[p][:],
                        start=(p == 0), stop=False,
                    )
                nc.tensor.matmul(
                    psf[:, 0:E], ones1[:], ob_row[:], start=False, stop=True,
                )
                fin = spool.tile([128, E], FP32, tag="fin", name=f"fin{m}")
                nc.scalar.activation(fin[:], psf[:, 0:E], Copy)
                nc.sync.dma_start(out[128 * m : 128 * m + 128, :], fin[:])

    return nc


_NC_CACHE = None


def _get_program():
    global _NC_CACHE
    if _NC_CACHE is None:
        _NC_CACHE = _build_program()
    return _NC_CACHE


def _make_in_maps(x, padding_mask, qkv_w, qkv_b, o_w, o_b):
    x = np.asarray(x, np.float32)
    pm = np.asarray(padding_mask)
    qkv_w = np.asarray(qkv_w, np.float32)
    qkv_b = np.asarray(qkv_b, np.float32)
    o_w = np.asarray(o_w, np.float32)
    o_b = np.asarray(o_b, np.float32)

    scale = np.float32(1.0 / np.sqrt(HD))
    # reference splits per-head: head h uses qkv rows [192h,192h+64) (q),
    # +64 (k), +128 (v)
    idx_q = np.concatenate([np.arange(3 * HD * h, 3 * HD * h + HD) for h in range(H)])
    idx_k = idx_q + HD
    idx_v = idx_q + 2 * HD

    wq = (qkv_w[idx_q] * scale).T.astype(BFNP)        # [IN, E]
    wk = qkv_w[idx_k].T.astype(BFNP)
    wv = qkv_w[idx_v].T.astype(BFNP)                  # [IN, E]
    # wv_ext: per head 64 cols + zero col (ones come from the bias row)
    wv_ext = np.zeros((IN_DIM, 520), BFNP)
    vb_ext = np.zeros((520,), BFNP)
    for h in range(H):
        wv_ext[:, 65 * h : 65 * h + 64] = wv[:, 64 * h : 64 * h + 64]
        vb_ext[65 * h : 65 * h + 64] = qkv_b[idx_v][64 * h : 64 * h + 64].astype(BFNP)
        vb_ext[65 * h + 64] = BFNP(1.0)
    ow = o_w.T.astype(BFNP)                           # [E_in, E_out]

    qkb = np.zeros((128, 8), np.float32)
    qkb[:, 0:4] = (qkv_b[idx_q] * scale).reshape(4, 128).T
    qkb[:, 4:8] = qkv_b[idx_k].reshape(4, 128).T

    rows2 = np.zeros((1, 1032), BFNP)
    rows2[0, 0:520] = vb_ext
    rows2[0, 520:1032] = o_b.astype(BFNP)


    # weights part of cat0/cat1/cat2 (identical for all cores)
    wq_cat = np.concatenate(
        [wq[128 * j : 128 * j + 128, :] for j in range(4)], axis=1
    )  # [128, 2048]
    cat1 = np.concatenate(
        [wk[128 * j : 128 * j + 128, :] for j in range(4)]
        + [wv_ext[128 * j : 128 * j + 128, :] for j in range(4)],
        axis=1,
    )  # [128, 4128]
    ow_cat = np.concatenate(
        [ow[128 * j : 128 * j + 128, :] for j in range(4)], axis=1
    )  # [128, 2048]

    j = np.arange(128)[:, None]
    in_maps = []
    for c in range(8):
        b, ch = divmod(c, 4)
        s0 = CH * ch
        lo, hi = max(0, s0 - HW), min(S, s0 + CH + HW)
        xpad = np.zeros((LK, IN_DIM), np.float32)
        xpad[lo - (s0 - HW) : hi - (s0 - HW)] = x[b, lo:hi]
        xt = np.ascontiguousarray(xpad.T).astype(BFNP)        # [IN, LK]

        mask = np.zeros((128, WSUM), np.float32)
        for t in range(NT):
            w, qs, off = W_T[t], QS_T[t], OFF_T[t]
            lk = 128 * t + j                                  # [128,1] local key
            q = qs + np.arange(w)[None, :]                    # [1,w] local query
            band = (q <= lk) & (lk <= q + 2 * HW)
            gk = s0 - HW + lk                                 # global key index
            valid = (gk >= 0) & (gk < S)
            pmk = pm[b, np.clip(gk, 0, S - 1)] != 0
            mask[:, off : off + w] = (band & valid & pmk).astype(np.float32)

        cat0 = np.concatenate(
            [xt[128 * k : 128 * k + 128, :] for k in range(4)] + [wq_cat], axis=1
        )  # [128, 5120]
        cat2 = np.concatenate([ow_cat, mask.astype(BFNP)], axis=1)

        in_maps.append(
            {
                "cat0": np.ascontiguousarray(cat0),
                "cat1": np.ascontiguousarray(cat1),
                "cat2": np.ascontiguousarray(cat2),
                "qkb": qkb,
                "rows2": rows2,
            }
        )
    return in_maps


def _run(x, padding_mask, qkv_w, qkv_b, o_w, o_b, trace=False, tmpdir=None):
    nc = _get_program()
    in_maps = _make_in_maps(x, padding_mask, qkv_w, qkv_b, o_w, o_b)
    res = run_bass_kernel_spmd(
        nc, in_maps, core_ids=list(range(8)), trace=trace, tmpdir=tmpdir
    )
    o = np.empty((B, S, E), np.float32)
    for c in range(8):
        b, ch = divmod(c, 4)
        o[b, CH * ch : CH * ch + CH, :] = res.results[c]["out"]
    # fully-masked query rows: att = 0 -> output is exactly the bias
    pm = np.asarray(padding_mask)
    if (pm == 0).any():
        o[pm == 0] = np.asarray(o_b, np.float32)
    return o, res


def kernel(x, padding_mask, qkv_w, qkv_b, o_w, o_b, window_size, num_heads):
    assert int(window_size) == WS and int(num_heads) == H
    assert tuple(np.asarray(x).shape) == (B, S, IN_DIM)
    o, _ = _run(x, padding_mask, qkv_w, qkv_b, o_w, o_b)
    return o


# revision 21
# speedup vs baseline: 1.3604x; 1.2090x over previous
"""Sliding-window multi-head attention for Trainium2, 8-core SPMD. v3.

Sharding: sequence-parallel. B=2 batches x 4 chunks of 512 queries = 8 cores.
Each core computes QKV projections for its chunk (+128-row halo for K/V),
banded attention (window 256 -> band |j-s|<=128), and the output projection
for its 512 rows. No collectives; host concatenates the 8 output chunks.

v3 vs v2 (89us): need-ordered split DMAs on both HWDGE queues (first matmul
~9us earlier), ACT-table preloaded at t0, masks all on DVE (GpSimd TT is
2.6cyc/elem AND port-starves the DVE when concurrent), whole-psO bf16 CAST
evacuation (kills the [1,512] den copies, makes vtn a 2x-rate bf16 op, and
frees psO banks early), bf16 denominator-broadcast matmuls, keep-warm dummy
matmuls bridging the last normalization chain so the output projection runs
at full clock.

Math notes (validated against the reference):
 - The reference's clamped scatter-add with zero-padded keys is exactly a
   banded score matrix: full[s,j] = q_s.k_j / 8 for |j-s|<=128, -inf outside.
 - Softmax computed without max-subtraction (scores are O(1), no overflow).
 - Denominators come free from the AV matmul via a ones-column on V (M=65).
 - Attention is computed transposed (scores^T[key, query]) so no transposes
   are needed anywhere in the hot loop.
"""

import numpy as np
import ml_dtypes

import concourse.bass as bass
import concourse.tile as tile
from concourse import mybir
from concourse.alu_op_type import AluOpType
from concourse.vector_clock import ScopedClock
from concourse.bass_utils import run_bass_kernel_spmd

FP32 = mybir.dt.float32
BF16 = mybir.dt.bfloat16
BFNP = ml_dtypes.bfloat16

# Problem constants (hardcoded per contract)
B, S, IN_DIM, E = 2, 2048, 512, 512
H, HD = 8, 64
WS, HW = 256, 128
CH = 512          # own queries per core
LK = 768          # local keys per core (chunk + 128 halo each side)
NT = 6            # key tiles of 128
W_T = [128, 256, 384, 384, 256, 128]   # valid query-span width per key tile
QS_T = [0, 0, 0, 128, 256, 384]        # local query start per key tile
OFF_T = [0, 128, 384, 768, 1152, 1408]  # column offset in the concat layout
WSUM = 1536
# QK segments split so no matmul output crosses a 512-col PSUM bank boundary:
# (key_tile, query_start, width, concat_offset)
SEGS = [(0, 0, 128, 0), (1, 0, 256, 128), (2, 0, 128, 384), (2, 128, 256, 512),
        (3, 128, 256, 768), (3, 384, 128, 1024), (4, 256, 256, 1152),
        (5, 384, 128, 1408)]

_MAX_WAITS = 1
_patched = False


def _split_sync_waits(nc):
    """This container's walrus accepts only 1 sync-wait per instruction.
    Move extra waits onto nofuse NOPs inserted just before, on the same
    engine sequencer (in-order execution makes this equivalent)."""
    n_split = 0
    for fn in nc.m.functions:
        for bb in fn.blocks:
            insts = list(bb.instructions)
            out = []
            for inst in insts:
                si = inst.sync_info
                if si is not None and len(si.on_wait) > _MAX_WAITS:
                    waits = list(si.on_wait)
                    extra, keep = waits[:-_MAX_WAITS], waits[-_MAX_WAITS:]
                    for j in range(0, len(extra), _MAX_WAITS):
                        out.append(
                            mybir.InstNoOp(
                                name=f"{inst.name}-sw{j}",
                                engine=inst.engine,
                                bass_nofuse=True,
                                sync_info=mybir.SyncInfo(
                                    on_wait=extra[j : j + _MAX_WAITS], on_update=[]
                                ),
                            )
                        )
                    inst.sync_info = mybir.SyncInfo(
                        on_wait=keep, on_update=list(si.on_update)
                    )
                    n_split += 1
                out.append(inst)
            if len(out) != len(insts):
                try:
                    bb.instructions = out
                except Exception:
                    bb.instructions[:] = out
    return n_split


def _patch_tile_drain():
    global _patched
    if _patched:
        return
    _patched = True

    def _drain_and_barrier(self, tick_clock, wait_clock):
        nc = self.nc
        drain_inst = nc.sync.drain()
        wait_clock.add_sem_waits(
            drain_inst.ins, ScopedClock({None: tick_clock.global_clock})
        )
        nc.all_engine_barrier()
        assert self.sems is not None
        popped = nc._tile_sem_poison_stack.pop()
        assert popped is self._sem_poison
        nc.clear_and_free_semaphores(list(self.sems.allocated().values()))
        nc.all_engine_barrier()
        _split_sync_waits(nc)

    tile.TileContext._drain_and_barrier = _drain_and_barrier


def _build_program():
    _patch_tile_drain()
    nc = bass.Bass("TRN2", target_bir_lowering=False, debug=False)

    d_xt = nc.dram_tensor("d_xt", [128, 4 * LK], BF16, kind="ExternalInput")
    d_wq = nc.dram_tensor("d_wq", [128, 2048], BF16, kind="ExternalInput")
    d_wk = nc.dram_tensor("d_wk", [128, 2048], BF16, kind="ExternalInput")
    d_wv = nc.dram_tensor("d_wv", [128, 2080], BF16, kind="ExternalInput")
    d_ow = nc.dram_tensor("d_ow", [128, 2048], BF16, kind="ExternalInput")
    d_mk = nc.dram_tensor("d_mk", [128, WSUM], BF16, kind="ExternalInput")
    qkb = nc.dram_tensor("qkb", [128, 8], FP32, kind="ExternalInput")
    rows2 = nc.dram_tensor("rows2", [1, 1032], BF16, kind="ExternalInput")
    out = nc.dram_tensor("out", [CH, E], FP32, kind="ExternalOutput")

    Exp = mybir.ActivationFunctionType.Exp
    Ln = mybir.ActivationFunctionType.Ln
    Ident = mybir.ActivationFunctionType.Identity
    Copy = mybir.ActivationFunctionType.Copy

    with tile.TileContext(nc) as tc:
        with (
            tc.tile_pool(name="const", bufs=1) as cpool,
            tc.tile_pool(name="proj", bufs=1) as ppool,
            tc.tile_pool(name="att", bufs=2) as apool,
            tc.tile_pool(name="small", bufs=2) as spool,
            tc.tile_pool(name="ps", bufs=2, space="PSUM") as ps,
        ):
            # ---- on-chip constants + ACT table preload (no DMA deps) ----
            ones1 = cpool.tile([1, 128], BF16, tag="ones1", name="ones1")
            nc.vector.memset(ones1[:], 1.0)
            # ones row at partition 64: broadcast-matmul lhsT must share its
            # base partition with the den row (psO row 64)
            ones64h = cpool.tile([65, 64], BF16, tag="ones64h", name="ones64h")
            nc.vector.memset(ones64h[64:65, :], 1.0)
            dum = cpool.tile([128, 512], BF16, tag="dum", name="dum")
            nc.vector.memset(dum[:], 0.0)
            scr = cpool.tile([1, 128], BF16, tag="scr", name="scr")
            nc.scalar.activation(scr[:], ones1[:], Exp)  # PWP table preload

            # ---- input DMAs, need-ordered across the two HWDGE queues ----
            xt_c = cpool.tile([128, 4 * LK], BF16, tag="xt", name="xt_c")
            wq_c = cpool.tile([128, 2048], BF16, tag="wq", name="wq_c")
            wk_c = cpool.tile([128, 2048], BF16, tag="wk", name="wk_c")
            wv_c = cpool.tile([128, 2080], BF16, tag="wv", name="wv_c")
            ow_c = cpool.tile([128, 2048], BF16, tag="ow", name="ow_c")
            mk_t = cpool.tile([128, WSUM], BF16, tag="mk", name="mk_t")
            qkb_t = cpool.tile([128, 8], FP32, tag="qkb", name="qkb_t")
            rows2_t = cpool.tile([1, 1032], BF16, tag="rows2", name="rows2_t")
            nc.scalar.dma_start(qkb_t[:], qkb[:])
            nc.scalar.dma_start(rows2_t[:], rows2[:])
            nc.sync.dma_start(xt_c[:], d_xt[:])
            nc.scalar.dma_start(wq_c[:], d_wq[:])
            nc.sync.dma_start(wv_c[:], d_wv[:])
            nc.scalar.dma_start(wk_c[:], d_wk[:])
            nc.sync.dma_start(mk_t[:], d_mk[:])
            nc.scalar.dma_start(ow_c[:], d_ow[:])

            xt_t = [xt_c[:, LK * k : LK * k + LK] for k in range(4)]
            wq_t = [wq_c[:, 512 * k : 512 * k + 512] for k in range(4)]
            wk_t = [wk_c[:, 512 * k : 512 * k + 512] for k in range(4)]
            wv_t = [wv_c[:, 520 * k : 520 * k + 520] for k in range(4)]
            ow_t = [ow_c[:, 512 * k : 512 * k + 512] for k in range(4)]
            vb_row = rows2_t[0:1, 0:520]
            ob_row = rows2_t[0:1, 520:1032]

            # HAM warmup: dummy matmuls with no DMA deps run while the input
            # DMAs stream in, so the PE clock gate is at 8/8 when the first
            # projection matmul issues. 'sc' tag slots have no user until QK0.
            for i in range(20):
                psd = ps.tile([128, 1536], FP32, tag="sc", name=f"dum{i}")
                nc.tensor.matmul(
                    psd[:, 0:512], dum[:, 0:128], dum[:, 0:512],
                    start=True, stop=True,
                )

            # ---- projections ----
            # qT[p]: [128 ch(2 heads), 512 q]; kT[p]: [128 ch, 768 keys]
            # biases applied on ACT during evacuation (per-partition bias AP)
            qT, kT = [None] * 4, [None] * 4
            for p in range(4):
                psq = ps.tile([128, 512], FP32, tag="ps1", name=f"psq{p}")
                for kk in range(4):
                    nc.tensor.matmul(
                        psq[:],
                        wq_t[kk][:, 128 * p : 128 * p + 128],
                        xt_t[kk][:, 128 : 128 + CH],
                        start=(kk == 0), stop=(kk == 3),
                    )
                q = ppool.tile([128, CH], BF16, tag=f"qT{p}", name=f"qT{p}")
                nc.scalar.activation(q[:], psq[:], Ident, bias=qkb_t[:, p : p + 1])
                qT[p] = q
                k_ = ppool.tile([128, LK], BF16, tag=f"kT{p}", name=f"kT{p}")
                kT[p] = k_
                for h, (c0, cw) in enumerate([(0, 512), (512, 256)]):
                    psk = ps.tile([128, cw], FP32, tag="ps1", name=f"psk{p}_{h}")
                    for kk in range(4):
                        nc.tensor.matmul(
                            psk[:],
                            wk_t[kk][:, 128 * p : 128 * p + 128],
                            xt_t[kk][:, c0 : c0 + cw],
                            start=(kk == 0), stop=(kk == 3),
                        )
                    nc.scalar.activation(
                        k_[:, c0 : c0 + cw], psk[:], Ident,
                        bias=qkb_t[:, 4 + p : 5 + p],
                    )

            # v in natural layout [keys, 8*(64+1)]: per head 64 v-cols + a
            # ones column (built by the K=1 bias-row matmul: wv_ext has zero
            # weight cols there and vb_row has 1.0) -> denominators free.
            v_t = []

            def emit_v(m):
                v = ppool.tile([128, 520], BF16, tag=f"v{m}", name=f"v{m}")
                for hh in range(2):
                    c0 = 260 * hh
                    psv = ps.tile([128, 260], FP32, tag="ps1", name=f"psv{m}_{hh}")
                    for kk in range(4):
                        nc.tensor.matmul(
                            psv[:],
                            xt_t[kk][:, 128 * m : 128 * m + 128],
                            wv_t[kk][:, c0 : c0 + 260],
                            start=(kk == 0), stop=False,
                        )
                    nc.tensor.matmul(
                        psv[:], ones1[:], vb_row[:, c0 : c0 + 260],
                        start=False, stop=True,
                    )
                    nc.vector.tensor_copy(v[:, c0 : c0 + 260], psv[:])
                v_t.append(v)

            for m in range(3):
                emit_v(m)

            # ---- attention ----
            att_tiles = {}
            pso_tiles = {}
            vals_tiles = {}
            rbs_tiles = {}
            vT = [None] * 4

            def emit_qk(p):
                # row-packed pairs: head A on PE rows 0-63, head B on 64-127,
                # concurrent via tile_position auto-derive from base_partition
                scA = ps.tile([128, WSUM], FP32, tag="sc", name=f"scA{p}")
                scB = ps.tile([128, WSUM], FP32, tag="sc", name=f"scB{p}")
                att_tiles[p] = (
                    apool.tile([128, WSUM], BF16, tag="attA", name=f"attA{p}"),
                    apool.tile([128, WSUM], BF16, tag="attB", name=f"attB{p}"),
                )
                for (t, qs, w, off) in SEGS:
                    nc.tensor.matmul(
                        scA[:, off : off + w],
                        kT[p][0:64, 128 * t : 128 * t + 128],
                        qT[p][0:64, qs : qs + w],
                        start=True, stop=True,
                    )
                    nc.tensor.matmul(
                        scB[:, off : off + w],
                        kT[p][64:128, 128 * t : 128 * t + 128],
                        qT[p][64:128, qs : qs + w],
                        start=True, stop=True,
                    )
                return scA, scB

            def emit_exp_masks(p, scA, scB):
                attA, attB = att_tiles[p]
                nc.scalar.activation(attA[:], scA[:], Exp)
                nc.scalar.activation(attB[:], scB[:], Exp)
                nc.vector.tensor_tensor(attA[:], attA[:], mk_t[:], op=AluOpType.mult)
                nc.vector.tensor_tensor(attB[:], attB[:], mk_t[:], op=AluOpType.mult)

            def emit_av(p):
                attA, attB = att_tiles[p]
                psoA = ps.tile([HD + 1, CH], FP32, tag="ps1", name=f"psoA{p}")
                psoB = ps.tile([HD + 1, CH], FP32, tag="ps1", name=f"psoB{p}")
                pso_tiles[p] = (psoA, psoB)
                for hh, (pso, att) in enumerate([(psoA, attA), (psoB, attB)]):
                    head = 2 * p + hh
                    for t in range(NT):
                        w, qs, off = W_T[t], QS_T[t], OFF_T[t]
                        nc.tensor.matmul(
                            pso[:, qs : qs + w],
                            v_t[t][:, 65 * head : 65 * head + 65],
                            att[:, off : off + w],
                            start=(t == 0), stop=(t == NT - 1),
                            skip_group_check=True,
                        )
                # evacuate head-A's psO (64 value rows + den row) as bf16 to
                # free its bank early; head-B's den row is copied alone (its
                # values are normalized straight from PSUM: a two-SBUF-input
                # op would need equal base partitions, which B cannot have)
                valsA = spool.tile([HD + 1, CH], BF16, tag="valsA", name=f"valsA{p}")
                denB = spool.tile([1, CH], BF16, tag="denB", name=f"denB{p}")
                nc.vector.tensor_copy(valsA[:], psoA[:])
                nc.vector.tensor_copy(denB[:], psoB[HD : HD + 1, :])
                vals_tiles[p] = (valsA, denB)

            def emit_rbc(p):
                # broadcast den rows to 64 partitions each via K=1 bf16
                # matmuls, then 1/x = exp(-ln x) on ACT
                valsA, denB = vals_tiles[p]
                rbc = ps.tile([128, CH], FP32, tag="sc", name=f"rbc{p}")
                nc.tensor.matmul(
                    rbc[0:64, :], ones64h[64:65, :], valsA[HD : HD + 1, :],
                    start=True, stop=True,
                )
                nc.tensor.matmul(
                    rbc[64:128, :], ones1[0:1, 0:64], denB[:],
                    start=True, stop=True, skip_group_check=True,
                )
                lnv = spool.tile([128, CH], FP32, tag="lnv", name=f"lnv{p}")
                nc.scalar.activation(lnv[:], rbc[:], Ln)
                rbs = spool.tile([128, CH], BF16, tag="rbs", name=f"rbs{p}")
                rbs_tiles[p] = rbs
                nc.scalar.activation(rbs[:], lnv[:], Exp, scale=-1.0)

            def emit_vtn(p):
                valsA, _ = vals_tiles[p]
                psoB = pso_tiles[p][1]
                rbs = rbs_tiles[p]
                vtn = ppool.tile([128, CH], BF16, tag=f"vT{p}", name=f"vT{p}")
                nc.vector.tensor_tensor(
                    vtn[0:64, :], valsA[0:HD, :], rbs[0:64, :], op=AluOpType.mult
                )
                nc.vector.tensor_tensor(
                    vtn[64:128, :], psoB[0:HD, :], rbs[64:128, :], op=AluOpType.mult
                )
                vT[p] = vtn

            # schedule: QK0, V3-5 (covers exp0 latency), QK1, AV0, QK2,
            # AV1, rbc0, vtn0, QK3, AV2, rbc1, vtn1, AV3, rbc2, vtn2, ...
            sc0 = emit_qk(0)
            emit_exp_masks(0, *sc0)
            for m in range(3, 6):
                emit_v(m)
            sc1 = emit_qk(1)
            emit_exp_masks(1, *sc1)
            emit_av(0)
            sc2 = emit_qk(2)
            emit_exp_masks(2, *sc2)
            emit_av(1)
            emit_rbc(0)
            emit_vtn(0)
            sc3 = emit_qk(3)
            emit_exp_masks(3, *sc3)
            emit_av(2)
            emit_rbc(1)
            emit_vtn(1)
            emit_av(3)
            emit_rbc(2)
            emit_vtn(2)
            emit_rbc(3)

            # ---- output projection (ob via K=1 bias row) ----
            # two 3-bank psf tiles hold all four 512-col m-chunks, so the
            # p=0..2 partial sums run while pair 3's normalization chain
            # finishes (bridges the would-be PE gap; HAM stays at 8/8)
            psfX = ps.tile([128, WSUM], FP32, tag="sc", name="psfX")
            psfY = ps.tile([128, WSUM], FP32, tag="sc", name="psfY")
            psf_m = [
                psfX[:, 0:512], psfX[:, 512:1024], psfX[:, 1024:1536],
                psfY[:, 0:512],
            ]
            for p in range(3):
                for m in range(4):
                    nc.tensor.matmul(
                        psf_m[m],
                        vT[p][:, 128 * m : 128 * m + 128],
                        ow_t[p][:],
                        start=(p == 0), stop=False, skip_group_check=True,
                    )
            emit_vtn(3)
            for m in range(4):
                nc.tensor.matmul(
                    psf_m[m],
                    vT[3][:, 128 * m : 128 * m + 128],
                    ow_t[3][:],
                    start=False, stop=False, skip_group_check=True,
                )
                nc.tensor.matmul(
                    psf_m[m], ones1[:], ob_row[:], start=False, stop=True,
                    skip_group_check=True,
                )
                fin = spool.tile([128, E], FP32, tag="fin", name=f"fin{m}")
                if m % 2 == 0:
                    nc.scalar.activation(fin[:], psf_m[m], Copy)
                else:
                    nc.vector.tensor_copy(fin[:], psf_m[m])
                nc.sync.dma_start(out[128 * m : 128 * m + 128, :], fin[:])

    return nc


_NC_CACHE = None


def _get_program():
    global _NC_CACHE
    if _NC_CACHE is None:
        _NC_CACHE = _build_program()
    return _NC_CACHE


def _make_in_maps(x, padding_mask, qkv_w, qkv_b, o_w, o_b):
    x = np.asarray(x, np.float32)
    pm = np.asarray(padding_mask)
    qkv_w = np.asarray(qkv_w, np.float32)
    qkv_b = np.asarray(qkv_b, np.float32)
    o_w = np.asarray(o_w, np.float32)
    o_b = np.asarray(o_b, np.float32)

    scale = np.float32(1.0 / np.sqrt(HD))
    # reference splits per-head: head h uses qkv rows [192h,192h+64) (q),
    # +64 (k), +128 (v)
    idx_q = np.concatenate([np.arange(3 * HD * h, 3 * HD * h + HD) for h in range(H)])
    idx_k = idx_q + HD
    idx_v = idx_q + 2 * HD

    wq = (qkv_w[idx_q] * scale).T.astype(BFNP)        # [IN, E]
    wk = qkv_w[idx_k].T.astype(BFNP)
    wv = qkv_w[idx_v].T.astype(BFNP)                  # [IN, E]
    # wv_ext: per head 64 cols + zero col (ones come from the bias row)
    wv_ext = np.zeros((IN_DIM, 520), BFNP)
    vb_ext = np.zeros((520,), BFNP)
    for h in range(H):
        wv_ext[:, 65 * h : 65 * h + 64] = wv[:, 64 * h : 64 * h + 64]
        vb_ext[65 * h : 65 * h + 64] = qkv_b[idx_v][64 * h : 64 * h + 64].astype(BFNP)
        vb_ext[65 * h + 64] = BFNP(1.0)
    ow = o_w.T.astype(BFNP)                           # [E_in, E_out]

    qkb = np.zeros((128, 8), np.float32)
    qkb[:, 0:4] = (qkv_b[idx_q] * scale).reshape(4, 128).T
    qkb[:, 4:8] = qkv_b[idx_k].reshape(4, 128).T

    rows2 = np.zeros((1, 1032), BFNP)
    rows2[0, 0:520] = vb_ext
    rows2[0, 520:1032] = o_b.astype(BFNP)

    # weight concats (identical for all cores): [128, 4*cols]
    cat = lambda w, c: np.ascontiguousarray(
        np.concatenate([w[128 * j : 128 * j + 128, :] for j in range(4)], axis=1)
    )
    d_wq, d_wk, d_ow = cat(wq, 512), cat(wk, 512), cat(ow, 512)
    d_wv = cat(wv_ext, 520)

    j = np.arange(128)[:, None]
    in_maps = []
    for c in range(8):
        b, ch = divmod(c, 4)
        s0 = CH * ch
        lo, hi = max(0, s0 - HW), min(S, s0 + CH + HW)
        xpad = np.zeros((LK, IN_DIM), np.float32)
        xpad[lo - (s0 - HW) : hi - (s0 - HW)] = x[b, lo:hi]
        xt = np.ascontiguousarray(xpad.T).astype(BFNP)        # [IN, LK]
        d_xt = np.ascontiguousarray(
            np.concatenate([xt[128 * k : 128 * k + 128, :] for k in range(4)], axis=1)
        )

        mask = np.zeros((128, WSUM), np.float32)
        for t in range(NT):
            w, qs, off = W_T[t], QS_T[t], OFF_T[t]
            lk = 128 * t + j                                  # [128,1] local key
            q = qs + np.arange(w)[None, :]                    # [1,w] local query
            band = (q <= lk) & (lk <= q + 2 * HW)
            gk = s0 - HW + lk                                 # global key index
            valid = (gk >= 0) & (gk < S)
            pmk = pm[b, np.clip(gk, 0, S - 1)] != 0
            mask[:, off : off + w] = (band & valid & pmk).astype(np.float32)

        in_maps.append(
            {
                "d_xt": d_xt,
                "d_wq": d_wq,
                "d_wk": d_wk,
                "d_wv": d_wv,
                "d_ow": d_ow,
                "d_mk": np.ascontiguousarray(mask.astype(BFNP)),
                "qkb": qkb,
                "rows2": rows2,
            }
        )
    return in_maps


def _run(x, padding_mask, qkv_w, qkv_b, o_w, o_b, trace=False, tmpdir=None):
    nc = _get_program()
    in_maps = _make_in_maps(x, padding_mask, qkv_w, qkv_b, o_w, o_b)
    res = run_bass_kernel_spmd(
        nc, in_maps, core_ids=list(range(8)), trace=trace, tmpdir=tmpdir
    )
    o = np.empty((B, S, E), np.float32)
    for c in range(8):
        b, ch = divmod(c, 4)
        o[b, CH * ch : CH * ch + CH, :] = res.results[c]["out"]
    # fully-masked query rows: att = 0 -> output is exactly the bias
    pm = np.asarray(padding_mask)
    if (pm == 0).any():
        o[pm == 0] = np.asarray(o_b, np.float32)
    return o, res


def kernel(x, padding_mask, qkv_w, qkv_b, o_w, o_b, window_size, num_heads):
    assert int(window_size) == WS and int(num_heads) == H
    assert tuple(np.asarray(x).shape) == (B, S, IN_DIM)
    o, _ = _run(x, padding_mask, qkv_w, qkv_b, o_w, o_b)
    return o


# revision 28
# speedup vs baseline: 1.3811x; 1.0152x over previous
"""Sliding-window multi-head attention for Trainium2, 8-core SPMD. v3.

Sharding: sequence-parallel. B=2 batches x 4 chunks of 512 queries = 8 cores.
Each core computes QKV projections for its chunk (+128-row halo for K/V),
banded attention (window 256 -> band |j-s|<=128), and the output projection
for its 512 rows. No collectives; host concatenates the 8 output chunks.

v3 vs v2 (89us): need-ordered split DMAs on both HWDGE queues (first matmul
~9us earlier), ACT-table preloaded at t0, masks all on DVE (GpSimd TT is
2.6cyc/elem AND port-starves the DVE when concurrent), whole-psO bf16 CAST
evacuation (kills the [1,512] den copies, makes vtn a 2x-rate bf16 op, and
frees psO banks early), bf16 denominator-broadcast matmuls, keep-warm dummy
matmuls bridging the last normalization chain so the output projection runs
at full clock.

Math notes (validated against the reference):
 - The reference's clamped scatter-add with zero-padded keys is exactly a
   banded score matrix: full[s,j] = q_s.k_j / 8 for |j-s|<=128, -inf outside.
 - Softmax computed without max-subtraction (scores are O(1), no overflow).
 - Denominators come free from the AV matmul via a ones-column on V (M=65).
 - Attention is computed transposed (scores^T[key, query]) so no transposes
   are needed anywhere in the hot loop.
"""

import numpy as np
import ml_dtypes

import concourse.bass as bass
import concourse.tile as tile
from concourse import mybir
from concourse.alu_op_type import AluOpType
from concourse.vector_clock import ScopedClock
from concourse.bass_utils import run_bass_kernel_spmd

FP32 = mybir.dt.float32
BF16 = mybir.dt.bfloat16
BFNP = ml_dtypes.bfloat16

# Problem constants (hardcoded per contract)
B, S, IN_DIM, E = 2, 2048, 512, 512
H, HD = 8, 64
WS, HW = 256, 128
CH = 512          # own queries per core
LK = 768          # local keys per core (chunk + 128 halo each side)
NT = 6            # key tiles of 128
W_T = [128, 256, 384, 384, 256, 128]   # valid query-span width per key tile
QS_T = [0, 0, 0, 128, 256, 384]        # local query start per key tile
OFF_T = [0, 128, 384, 768, 1152, 1408]  # column offset in the concat layout
WSUM = 1536
# QK segments split so no matmul output crosses a 512-col PSUM bank boundary:
# (key_tile, query_start, width, concat_offset)
SEGS = [(0, 0, 128, 0), (1, 0, 256, 128), (2, 0, 128, 384), (2, 128, 256, 512),
        (3, 128, 256, 768), (3, 384, 128, 1024), (4, 256, 256, 1152),
        (5, 384, 128, 1408)]

_MAX_WAITS = 1
_patched = False


def _split_sync_waits(nc):
    """This container's walrus accepts only 1 sync-wait per instruction.
    Move extra waits onto nofuse NOPs inserted just before, on the same
    engine sequencer (in-order execution makes this equivalent)."""
    n_split = 0
    for fn in nc.m.functions:
        for bb in fn.blocks:
            insts = list(bb.instructions)
            out = []
            for inst in insts:
                si = inst.sync_info
                if si is not None and len(si.on_wait) > _MAX_WAITS:
                    waits = list(si.on_wait)
                    extra, keep = waits[:-_MAX_WAITS], waits[-_MAX_WAITS:]
                    for j in range(0, len(extra), _MAX_WAITS):
                        out.append(
                            mybir.InstNoOp(
                                name=f"{inst.name}-sw{j}",
                                engine=inst.engine,
                                bass_nofuse=True,
                                sync_info=mybir.SyncInfo(
                                    on_wait=extra[j : j + _MAX_WAITS], on_update=[]
                                ),
                            )
                        )
                    inst.sync_info = mybir.SyncInfo(
                        on_wait=keep, on_update=list(si.on_update)
                    )
                    n_split += 1
                out.append(inst)
            if len(out) != len(insts):
                try:
                    bb.instructions = out
                except Exception:
                    bb.instructions[:] = out
    return n_split


def _patch_tile_drain():
    global _patched
    if _patched:
        return
    _patched = True

    def _drain_and_barrier(self, tick_clock, wait_clock):
        nc = self.nc
        drain_inst = nc.sync.drain()
        wait_clock.add_sem_waits(
            drain_inst.ins, ScopedClock({None: tick_clock.global_clock})
        )
        nc.all_engine_barrier()
        assert self.sems is not None
        popped = nc._tile_sem_poison_stack.pop()
        assert popped is self._sem_poison
        nc.clear_and_free_semaphores(list(self.sems.allocated().values()))
        nc.all_engine_barrier()
        _split_sync_waits(nc)

    tile.TileContext._drain_and_barrier = _drain_and_barrier


def _build_program():
    _patch_tile_drain()
    nc = bass.Bass("TRN2", target_bir_lowering=False, debug=False)

    d_xt = nc.dram_tensor("d_xt", [128, 4 * LK], BF16, kind="ExternalInput")
    d_wq = nc.dram_tensor("d_wq", [128, 2048], BF16, kind="ExternalInput")
    d_wk = nc.dram_tensor("d_wk", [128, 2048], BF16, kind="ExternalInput")
    d_wv = nc.dram_tensor("d_wv", [128, 2080], BF16, kind="ExternalInput")
    d_ow = nc.dram_tensor("d_ow", [128, 2048], BF16, kind="ExternalInput")
    d_mk = nc.dram_tensor("d_mk", [128, WSUM], BF16, kind="ExternalInput")
    qkb = nc.dram_tensor("qkb", [128, 8], FP32, kind="ExternalInput")
    rows2 = nc.dram_tensor("rows2", [1, 1032], BF16, kind="ExternalInput")
    out = nc.dram_tensor("out", [CH, E], FP32, kind="ExternalOutput")

    Exp = mybir.ActivationFunctionType.Exp
    Ln = mybir.ActivationFunctionType.Ln
    Ident = mybir.ActivationFunctionType.Identity
    Copy = mybir.ActivationFunctionType.Copy

    with tile.TileContext(nc) as tc:
        with (
            tc.tile_pool(name="const", bufs=1) as cpool,
            tc.tile_pool(name="proj", bufs=1) as ppool,
            tc.tile_pool(name="att", bufs=2) as apool,
            tc.tile_pool(name="small", bufs=2) as spool,
            tc.tile_pool(name="ps", bufs=2, space="PSUM") as ps,
        ):
            # ---- on-chip constants + ACT table preload (no DMA deps) ----
            ones1 = cpool.tile([1, 128], BF16, tag="ones1", name="ones1")
            nc.vector.memset(ones1[:], 1.0)
            # ones row at partition 64: broadcast-matmul lhsT must share its
            # base partition with the den row (psO row 64)
            ones64h = cpool.tile([65, 64], BF16, tag="ones64h", name="ones64h")
            nc.vector.memset(ones64h[64:65, :], 1.0)
            dum = cpool.tile([128, 512], BF16, tag="dum", name="dum")
            nc.vector.memset(dum[:], 0.0)
            scr = cpool.tile([1, 128], BF16, tag="scr", name="scr")
            nc.scalar.activation(scr[:], ones1[:], Exp)  # PWP table preload

            # ---- input DMAs, need-ordered across the two HWDGE queues ----
            xt_c = cpool.tile([128, 4 * LK], BF16, tag="xt", name="xt_c")
            wq_c = cpool.tile([128, 2048], BF16, tag="wq", name="wq_c")
            wk_c = cpool.tile([128, 2048], BF16, tag="wk", name="wk_c")
            wv_c = cpool.tile([128, 2080], BF16, tag="wv", name="wv_c")
            ow_c = cpool.tile([128, 2048], BF16, tag="ow", name="ow_c")
            mk_t = cpool.tile([128, WSUM], BF16, tag="mk", name="mk_t")
            qkb_t = cpool.tile([128, 8], FP32, tag="qkb", name="qkb_t")
            rows2_t = cpool.tile([1, 1032], BF16, tag="rows2", name="rows2_t")
            nc.scalar.dma_start(qkb_t[:], qkb[:])
            nc.scalar.dma_start(rows2_t[:], rows2[:])
            nc.sync.dma_start(xt_c[:], d_xt[:])
            nc.scalar.dma_start(wq_c[:], d_wq[:])
            nc.sync.dma_start(wv_c[:], d_wv[:])
            nc.scalar.dma_start(wk_c[:], d_wk[:])
            nc.sync.dma_start(mk_t[:], d_mk[:])
            nc.scalar.dma_start(ow_c[:], d_ow[:])

            xt_t = [xt_c[:, LK * k : LK * k + LK] for k in range(4)]
            wq_t = [wq_c[:, 512 * k : 512 * k + 512] for k in range(4)]
            wk_t = [wk_c[:, 512 * k : 512 * k + 512] for k in range(4)]
            wv_t = [wv_c[:, 520 * k : 520 * k + 520] for k in range(4)]
            ow_t = [ow_c[:, 512 * k : 512 * k + 512] for k in range(4)]
            vb_row = rows2_t[0:1, 0:520]
            ob_row = rows2_t[0:1, 520:1032]

            # HAM warmup: dummy matmuls with no DMA deps run while the input
            # DMAs stream in, so the PE clock gate is at 8/8 when the first
            # projection matmul issues. 'sc' tag slots have no user until QK0.
            for i in range(34):
                psd = ps.tile([128, 1536], FP32, tag="sc", name=f"dum{i}")
                nc.tensor.matmul(
                    psd[:, 0:512], dum[:, 0:128], dum[:, 0:512],
                    start=True, stop=True,
                )

            def keep_warm(tag, n):
                # no-dep dummy matmuls fill otherwise-idle PE slots so the
                # HAM activity monitor never re-throttles the clock to 4/8
                for i in range(n):
                    psd = ps.tile([128, 512], FP32, tag="ps1", name=f"kw_{tag}_{i}")
                    nc.tensor.matmul(
                        psd[:], dum[:, 0:128], dum[:, 0:512], start=True, stop=True,
                    )

            # ---- projections ----
            # qT[p]: [128 ch(2 heads), 512 q]; kT[p]: [128 ch, 768 keys]
            # biases applied on ACT during evacuation (per-partition bias AP)
            qT, kT = [None] * 4, [None] * 4
            for p in range(4):
                psq = ps.tile([128, 512], FP32, tag="ps1", name=f"psq{p}")
                for kk in range(4):
                    nc.tensor.matmul(
                        psq[:],
                        wq_t[kk][:, 128 * p : 128 * p + 128],
                        xt_t[kk][:, 128 : 128 + CH],
                        start=(kk == 0), stop=(kk == 3),
                    )
                q = ppool.tile([128, CH], BF16, tag=f"qT{p}", name=f"qT{p}")
                nc.scalar.activation(q[:], psq[:], Ident, bias=qkb_t[:, p : p + 1])
                qT[p] = q
                k_ = ppool.tile([128, LK], BF16, tag=f"kT{p}", name=f"kT{p}")
                kT[p] = k_
                for h, (c0, cw) in enumerate([(0, 512), (512, 256)]):
                    psk = ps.tile([128, cw], FP32, tag="ps1", name=f"psk{p}_{h}")
                    for kk in range(4):
                        nc.tensor.matmul(
                            psk[:],
                            wk_t[kk][:, 128 * p : 128 * p + 128],
                            xt_t[kk][:, c0 : c0 + cw],
                            start=(kk == 0), stop=(kk == 3),
                        )
                    nc.scalar.activation(
                        k_[:, c0 : c0 + cw], psk[:], Ident,
                        bias=qkb_t[:, 4 + p : 5 + p],
                    )

            # v in natural layout [keys, 8*(64+1)]: per head 64 v-cols + a
            # ones column (built by the K=1 bias-row matmul: wv_ext has zero
            # weight cols there and vb_row has 1.0) -> denominators free.
            v_t = []

            def emit_v(m):
                v = ppool.tile([128, 520], BF16, tag=f"v{m}", name=f"v{m}")
                for hh in range(2):
                    c0 = 260 * hh
                    psv = ps.tile([128, 260], FP32, tag="ps1", name=f"psv{m}_{hh}")
                    for kk in range(4):
                        nc.tensor.matmul(
                            psv[:],
                            xt_t[kk][:, 128 * m : 128 * m + 128],
                            wv_t[kk][:, c0 : c0 + 260],
                            start=(kk == 0), stop=False,
                        )
                    nc.tensor.matmul(
                        psv[:], ones1[:], vb_row[:, c0 : c0 + 260],
                        start=False, stop=True,
                    )
                    # split evacuations across ACT and DVE (both ~idle here)
                    if (m + hh) % 2 == 0:
                        nc.vector.tensor_copy(v[:, c0 : c0 + 260], psv[:])
                    else:
                        nc.scalar.activation(v[:, c0 : c0 + 260], psv[:], Copy)
                v_t.append(v)

            for m in range(3):
                emit_v(m)

            # ---- attention ----
            att_tiles = {}
            pso_tiles = {}
            vals_tiles = {}
            rbs_tiles = {}
            vT = [None] * 4

            def emit_qk(p):
                # row-packed pairs: head A on PE rows 0-63, head B on 64-127,
                # concurrent via tile_position auto-derive from base_partition
                scA = ps.tile([128, WSUM], FP32, tag="sc", name=f"scA{p}")
                scB = ps.tile([128, WSUM], FP32, tag="sc", name=f"scB{p}")
                att_tiles[p] = (
                    apool.tile([128, WSUM], BF16, tag="attA", name=f"attA{p}"),
                    apool.tile([128, WSUM], BF16, tag="attB", name=f"attB{p}"),
                )
                for (t, qs, w, off) in SEGS:
                    nc.tensor.matmul(
                        scA[:, off : off + w],
                        kT[p][0:64, 128 * t : 128 * t + 128],
                        qT[p][0:64, qs : qs + w],
                        start=True, stop=True,
                    )
                    nc.tensor.matmul(
                        scB[:, off : off + w],
                        kT[p][64:128, 128 * t : 128 * t + 128],
                        qT[p][64:128, qs : qs + w],
                        start=True, stop=True,
                    )
                return scA, scB

            def emit_exp_masks(p, scA, scB):
                attA, attB = att_tiles[p]
                nc.scalar.activation(attA[:], scA[:], Exp)
                nc.scalar.activation(attB[:], scB[:], Exp)
                nc.vector.tensor_tensor(attA[:], attA[:], mk_t[:], op=AluOpType.mult)
                nc.vector.tensor_tensor(attB[:], attB[:], mk_t[:], op=AluOpType.mult)

            def emit_av(p):
                attA, attB = att_tiles[p]
                psoA = ps.tile([HD + 1, CH], FP32, tag="ps1", name=f"psoA{p}")
                psoB = ps.tile([HD + 1, CH], FP32, tag="ps1", name=f"psoB{p}")
                pso_tiles[p] = (psoA, psoB)
                for hh, (pso, att) in enumerate([(psoA, attA), (psoB, attB)]):
                    head = 2 * p + hh
                    for t in range(NT):
                        w, qs, off = W_T[t], QS_T[t], OFF_T[t]
                        nc.tensor.matmul(
                            pso[:, qs : qs + w],
                            v_t[t][:, 65 * head : 65 * head + 65],
                            att[:, off : off + w],
                            start=(t == 0), stop=(t == NT - 1),
                            skip_group_check=True,
                        )
                # evacuate both psO tiles (64 value rows + den row) as bf16:
                # frees the psO banks immediately, so pair pipelining is
                # gated only by these fast copies, not the recip chain
                valsA = spool.tile([HD + 1, CH], BF16, tag="valsA", name=f"valsA{p}")
                valsB = spool.tile([HD + 1, CH], BF16, tag="valsB", name=f"valsB{p}")
                nc.vector.tensor_copy(valsA[:], psoA[:])
                nc.vector.tensor_copy(valsB[:], psoB[:])
                # DVE lanes cannot shift partitions; DMA-hop B's values to
                # partitions 64:128 so vtn_B is a lane-aligned two-SBUF op
                vB64 = spool.tile([128, CH], BF16, tag="vB64", name=f"vB64_{p}")
                nc.scalar.dma_start(vB64[64:128, :], valsB[0:HD, :])
                vals_tiles[p] = (valsA, valsB, vB64)

            def emit_rbc(p):
                # broadcast den rows to 64 partitions each via K=1 bf16
                # matmuls, then 1/x = exp(-ln x) on ACT. The B-half recip
                # (partitions 64:128) is DMA-hopped to a base-0 tile so vtn
                # can be a two-SBUF-input op (equal-base-partition rule).
                valsA, valsB, _ = vals_tiles[p]
                rbc = ps.tile([128, CH], FP32, tag="sc", name=f"rbc{p}")
                nc.tensor.matmul(
                    rbc[0:64, :], ones64h[64:65, :], valsA[HD : HD + 1, :],
                    start=True, stop=True,
                )
                nc.tensor.matmul(
                    rbc[64:128, :], ones64h[64:65, :], valsB[HD : HD + 1, :],
                    start=True, stop=True, skip_group_check=True,
                )
                lnv = spool.tile([128, CH], FP32, tag="lnv", name=f"lnv{p}")
                nc.scalar.activation(lnv[:], rbc[:], Ln)
                rbs = spool.tile([128, CH], BF16, tag="rbs", name=f"rbs{p}")
                nc.scalar.activation(rbs[:], lnv[:], Exp, scale=-1.0)
                rbs_tiles[p] = rbs

            def emit_vtn(p):
                valsA, _, vB64 = vals_tiles[p]
                rbs = rbs_tiles[p]
                vtn = ppool.tile([128, CH], BF16, tag=f"vT{p}", name=f"vT{p}")
                nc.vector.tensor_tensor(
                    vtn[0:64, :], valsA[0:HD, :], rbs[0:64, :], op=AluOpType.mult
                )
                nc.vector.tensor_tensor(
                    vtn[64:128, :], vB64[64:128, :], rbs[64:128, :], op=AluOpType.mult
                )
                vT[p] = vtn

            # schedule: QK0, V3-5 (covers exp0 latency), QK1, AV0, QK2,
            # AV1, rbc0, vtn0, QK3, AV2, rbc1, vtn1, AV3, rbc2, vtn2, ...
            sc0 = emit_qk(0)
            emit_exp_masks(0, *sc0)
            for m in range(3, 6):
                emit_v(m)
            sc1 = emit_qk(1)
            emit_exp_masks(1, *sc1)
            emit_av(0)
            keep_warm("a", 5)
            sc2 = emit_qk(2)
            emit_exp_masks(2, *sc2)
            emit_av(1)
            emit_rbc(0)
            emit_vtn(0)
            keep_warm("b", 5)
            sc3 = emit_qk(3)
            emit_exp_masks(3, *sc3)
            emit_av(2)
            emit_rbc(1)
            emit_vtn(1)
            keep_warm("c", 5)
            emit_av(3)
            emit_rbc(2)
            emit_vtn(2)
            emit_rbc(3)
            keep_warm("d", 4)

            # ---- output projection (ob via K=1 bias row) ----
            # two 3-bank psf tiles hold all four 512-col m-chunks, so the
            # p=0..2 partial sums run while pair 3's normalization chain
            # finishes (bridges the would-be PE gap; HAM stays at 8/8)
            psfX = ps.tile([128, WSUM], FP32, tag="sc", name="psfX")
            psfY = ps.tile([128, WSUM], FP32, tag="sc", name="psfY")
            psf_m = [
                psfX[:, 0:512], psfX[:, 512:1024], psfX[:, 1024:1536],
                psfY[:, 0:512],
            ]
            for p in range(3):
                for m in range(4):
                    nc.tensor.matmul(
                        psf_m[m],
                        vT[p][:, 128 * m : 128 * m + 128],
                        ow_t[p][:],
                        start=(p == 0), stop=False, skip_group_check=True,
                    )
            emit_vtn(3)
            for m in range(4):
                nc.tensor.matmul(
                    psf_m[m],
                    vT[3][:, 128 * m : 128 * m + 128],
                    ow_t[3][:],
                    start=False, stop=False, skip_group_check=True,
                )
                nc.tensor.matmul(
                    psf_m[m], ones1[:], ob_row[:], start=False, stop=True,
                    skip_group_check=True,
                )
                fin = spool.tile([128, E], FP32, tag="fin", name=f"fin{m}")
                if m % 2 == 0:
                    nc.scalar.activation(fin[:], psf_m[m], Copy)
                    nc.sync.dma_start(out[128 * m : 128 * m + 128, :], fin[:])
                else:
                    nc.vector.tensor_copy(fin[:], psf_m[m])
                    nc.scalar.dma_start(out[128 * m : 128 * m + 128, :], fin[:])

    return nc


_NC_CACHE = None


def _get_program():
    global _NC_CACHE
    if _NC_CACHE is None:
        _NC_CACHE = _build_program()
    return _NC_CACHE


def _make_in_maps(x, padding_mask, qkv_w, qkv_b, o_w, o_b):
    x = np.asarray(x, np.float32)
    pm = np.asarray(padding_mask)
    qkv_w = np.asarray(qkv_w, np.float32)
    qkv_b = np.asarray(qkv_b, np.float32)
    o_w = np.asarray(o_w, np.float32)
    o_b = np.asarray(o_b, np.float32)

    scale = np.float32(1.0 / np.sqrt(HD))
    # reference splits per-head: head h uses qkv rows [192h,192h+64) (q),
    # +64 (k), +128 (v)
    idx_q = np.concatenate([np.arange(3 * HD * h, 3 * HD * h + HD) for h in range(H)])
    idx_k = idx_q + HD
    idx_v = idx_q + 2 * HD

    wq = (qkv_w[idx_q] * scale).T.astype(BFNP)        # [IN, E]
    wk = qkv_w[idx_k].T.astype(BFNP)
    wv = qkv_w[idx_v].T.astype(BFNP)                  # [IN, E]
    # wv_ext: per head 64 cols + zero col (ones come from the bias row)
    wv_ext = np.zeros((IN_DIM, 520), BFNP)
    vb_ext = np.zeros((520,), BFNP)
    for h in range(H):
        wv_ext[:, 65 * h : 65 * h + 64] = wv[:, 64 * h : 64 * h + 64]
        vb_ext[65 * h : 65 * h + 64] = qkv_b[idx_v][64 * h : 64 * h + 64].astype(BFNP)
        vb_ext[65 * h + 64] = BFNP(1.0)
    ow = o_w.T.astype(BFNP)                           # [E_in, E_out]

    qkb = np.zeros((128, 8), np.float32)
    qkb[:, 0:4] = (qkv_b[idx_q] * scale).reshape(4, 128).T
    qkb[:, 4:8] = qkv_b[idx_k].reshape(4, 128).T

    rows2 = np.zeros((1, 1032), BFNP)
    rows2[0, 0:520] = vb_ext
    rows2[0, 520:1032] = o_b.astype(BFNP)

    # weight concats (identical for all cores): [128, 4*cols]
    cat = lambda w, c: np.ascontiguousarray(
        np.concatenate([w[128 * j : 128 * j + 128, :] for j in range(4)], axis=1)
    )
    d_wq, d_wk, d_ow = cat(wq, 512), cat(wk, 512), cat(ow, 512)
    d_wv = cat(wv_ext, 520)

    j = np.arange(128)[:, None]
    in_maps = []
    for c in range(8):
        b, ch = divmod(c, 4)
        s0 = CH * ch
        lo, hi = max(0, s0 - HW), min(S, s0 + CH + HW)
        xpad = np.zeros((LK, IN_DIM), np.float32)
        xpad[lo - (s0 - HW) : hi - (s0 - HW)] = x[b, lo:hi]
        xt = np.ascontiguousarray(xpad.T).astype(BFNP)        # [IN, LK]
        d_xt = np.ascontiguousarray(
            np.concatenate([xt[128 * k : 128 * k + 128, :] for k in range(4)], axis=1)
        )

        mask = np.zeros((128, WSUM), np.float32)
        for t in range(NT):
            w, qs, off = W_T[t], QS_T[t], OFF_T[t]
            lk = 128 * t + j                                  # [128,1] local key
            q = qs + np.arange(w)[None, :]                    # [1,w] local query
            band = (q <= lk) & (lk <= q + 2 * HW)
            gk = s0 - HW + lk                                 # global key index
            valid = (gk >= 0) & (gk < S)
            pmk = pm[b, np.clip(gk, 0, S - 1)] != 0
            mask[:, off : off + w] = (band & valid & pmk).astype(np.float32)

        in_maps.append(
            {
                "d_xt": d_xt,
                "d_wq": d_wq,
                "d_wk": d_wk,
                "d_wv": d_wv,
                "d_ow": d_ow,
                "d_mk": np.ascontiguousarray(mask.astype(BFNP)),
                "qkb": qkb,
                "rows2": rows2,
            }
        )
    return in_maps


def _run(x, padding_mask, qkv_w, qkv_b, o_w, o_b, trace=False, tmpdir=None):
    nc = _get_program()
    in_maps = _make_in_maps(x, padding_mask, qkv_w, qkv_b, o_w, o_b)
    res = run_bass_kernel_spmd(
        nc, in_maps, core_ids=list(range(8)), trace=trace, tmpdir=tmpdir
    )
    o = np.empty((B, S, E), np.float32)
    for c in range(8):
        b, ch = divmod(c, 4)
        o[b, CH * ch : CH * ch + CH, :] = res.results[c]["out"]
    # fully-masked query rows: att = 0 -> output is exactly the bias
    pm = np.asarray(padding_mask)
    if (pm == 0).any():
        o[pm == 0] = np.asarray(o_b, np.float32)
    return o, res


def kernel(x, padding_mask, qkv_w, qkv_b, o_w, o_b, window_size, num_heads):
    assert int(window_size) == WS and int(num_heads) == H
    assert tuple(np.asarray(x).shape) == (B, S, IN_DIM)
    o, _ = _run(x, padding_mask, qkv_w, qkv_b, o_w, o_b)
    return o


# revision 30
# speedup vs baseline: 1.5512x; 1.1231x over previous
"""Sliding-window multi-head attention for Trainium2, 8-core SPMD. v3.

Sharding: sequence-parallel. B=2 batches x 4 chunks of 512 queries = 8 cores.
Each core computes QKV projections for its chunk (+128-row halo for K/V),
banded attention (window 256 -> band |j-s|<=128), and the output projection
for its 512 rows. No collectives; host concatenates the 8 output chunks.

v3 vs v2 (89us): need-ordered split DMAs on both HWDGE queues (first matmul
~9us earlier), ACT-table preloaded at t0, masks all on DVE (GpSimd TT is
2.6cyc/elem AND port-starves the DVE when concurrent), whole-psO bf16 CAST
evacuation (kills the [1,512] den copies, makes vtn a 2x-rate bf16 op, and
frees psO banks early), bf16 denominator-broadcast matmuls, keep-warm dummy
matmuls bridging the last normalization chain so the output projection runs
at full clock.

Math notes (validated against the reference):
 - The reference's clamped scatter-add with zero-padded keys is exactly a
   banded score matrix: full[s,j] = q_s.k_j / 8 for |j-s|<=128, -inf outside.
 - Softmax computed without max-subtraction (scores are O(1), no overflow).
 - Denominators come free from the AV matmul via a ones-column on V (M=65).
 - Attention is computed transposed (scores^T[key, query]) so no transposes
   are needed anywhere in the hot loop.
"""

import numpy as np
import ml_dtypes

import concourse.bass as bass
import concourse.tile as tile
from concourse import mybir
from concourse.alu_op_type import AluOpType
from concourse.vector_clock import ScopedClock
from concourse.bass_utils import run_bass_kernel_spmd

FP32 = mybir.dt.float32
BF16 = mybir.dt.bfloat16
BFNP = ml_dtypes.bfloat16

# Problem constants (hardcoded per contract)
B, S, IN_DIM, E = 2, 2048, 512, 512
H, HD = 8, 64
WS, HW = 256, 128
CH = 512          # own queries per core
LK = 768          # local keys per core (chunk + 128 halo each side)
NT = 6            # key tiles of 128
W_T = [128, 256, 384, 384, 256, 128]   # valid query-span width per key tile
QS_T = [0, 0, 0, 128, 256, 384]        # local query start per key tile
OFF_T = [0, 128, 384, 768, 1152, 1408]  # column offset in the concat layout
WSUM = 1536
# QK segments split so no matmul output crosses a 512-col PSUM bank boundary:
# (key_tile, query_start, width, concat_offset)
SEGS = [(0, 0, 128, 0), (1, 0, 256, 128), (2, 0, 128, 384), (2, 128, 256, 512),
        (3, 128, 256, 768), (3, 384, 128, 1024), (4, 256, 256, 1152),
        (5, 384, 128, 1408)]

_MAX_WAITS = 1
_patched = False


def _split_sync_waits(nc):
    """This container's walrus accepts only 1 sync-wait per instruction.
    Move extra waits onto nofuse NOPs inserted just before, on the same
    engine sequencer (in-order execution makes this equivalent)."""
    n_split = 0
    for fn in nc.m.functions:
        for bb in fn.blocks:
            insts = list(bb.instructions)
            out = []
            for inst in insts:
                si = inst.sync_info
                if si is not None and len(si.on_wait) > _MAX_WAITS:
                    waits = list(si.on_wait)
                    extra, keep = waits[:-_MAX_WAITS], waits[-_MAX_WAITS:]
                    for j in range(0, len(extra), _MAX_WAITS):
                        out.append(
                            mybir.InstNoOp(
                                name=f"{inst.name}-sw{j}",
                                engine=inst.engine,
                                bass_nofuse=True,
                                sync_info=mybir.SyncInfo(
                                    on_wait=extra[j : j + _MAX_WAITS], on_update=[]
                                ),
                            )
                        )
                    inst.sync_info = mybir.SyncInfo(
                        on_wait=keep, on_update=list(si.on_update)
                    )
                    n_split += 1
                out.append(inst)
            if len(out) != len(insts):
                try:
                    bb.instructions = out
                except Exception:
                    bb.instructions[:] = out
    return n_split


def _patch_tile_drain():
    global _patched
    if _patched:
        return
    _patched = True

    def _drain_and_barrier(self, tick_clock, wait_clock):
        nc = self.nc
        drain_inst = nc.sync.drain()
        wait_clock.add_sem_waits(
            drain_inst.ins, ScopedClock({None: tick_clock.global_clock})
        )
        nc.all_engine_barrier()
        assert self.sems is not None
        popped = nc._tile_sem_poison_stack.pop()
        assert popped is self._sem_poison
        nc.clear_and_free_semaphores(list(self.sems.allocated().values()))
        nc.all_engine_barrier()
        _split_sync_waits(nc)

    tile.TileContext._drain_and_barrier = _drain_and_barrier


def _build_program():
    _patch_tile_drain()
    nc = bass.Bass("TRN2", target_bir_lowering=False, debug=False)

    d_xt = nc.dram_tensor("d_xt", [128, 4 * LK], BF16, kind="ExternalInput")
    d_wq = nc.dram_tensor("d_wq", [128, 2048], BF16, kind="ExternalInput")
    d_wk = nc.dram_tensor("d_wk", [128, 2048], BF16, kind="ExternalInput")
    d_wv = nc.dram_tensor("d_wv", [128, 2080], BF16, kind="ExternalInput")
    d_ow = nc.dram_tensor("d_ow", [128, 2048], BF16, kind="ExternalInput")
    d_mk = nc.dram_tensor("d_mk", [128, WSUM], BF16, kind="ExternalInput")
    qkb = nc.dram_tensor("qkb", [128, 8], FP32, kind="ExternalInput")
    rows2 = nc.dram_tensor("rows2", [1, 1032], BF16, kind="ExternalInput")
    out = nc.dram_tensor("out", [CH, E], FP32, kind="ExternalOutput")

    Exp = mybir.ActivationFunctionType.Exp
    Ln = mybir.ActivationFunctionType.Ln
    Ident = mybir.ActivationFunctionType.Identity
    Copy = mybir.ActivationFunctionType.Copy

    with tile.TileContext(nc) as tc:
        with (
            tc.tile_pool(name="const", bufs=1) as cpool,
            tc.tile_pool(name="proj", bufs=1) as ppool,
            tc.tile_pool(name="att", bufs=4) as apool,
            tc.tile_pool(name="small", bufs=2) as spool,
            tc.tile_pool(name="ps", bufs=2, space="PSUM") as ps,
        ):
            # ---- on-chip constants + ACT table preload (no DMA deps) ----
            ones1 = cpool.tile([1, 128], BF16, tag="ones1", name="ones1")
            nc.vector.memset(ones1[:], 1.0)
            # ones row at partition 64: broadcast-matmul lhsT must share its
            # base partition with the den row (psO row 64)
            ones64h = cpool.tile([65, 64], BF16, tag="ones64h", name="ones64h")
            nc.vector.memset(ones64h[64:65, :], 1.0)
            dum = cpool.tile([128, 512], BF16, tag="dum", name="dum")
            nc.vector.memset(dum[:], 0.0)
            scr = cpool.tile([1, 128], BF16, tag="scr", name="scr")
            nc.scalar.activation(scr[:], ones1[:], Exp)  # PWP table preload

            # ---- input DMAs, need-ordered across the two HWDGE queues ----
            xt_c = cpool.tile([128, 4 * LK], BF16, tag="xt", name="xt_c")
            wq_c = cpool.tile([128, 2048], BF16, tag="wq", name="wq_c")
            wk_c = cpool.tile([128, 2048], BF16, tag="wk", name="wk_c")
            wv_c = cpool.tile([128, 2080], BF16, tag="wv", name="wv_c")
            ow_c = cpool.tile([128, 2048], BF16, tag="ow", name="ow_c")
            mk_t = cpool.tile([128, WSUM], BF16, tag="mk", name="mk_t")
            qkb_t = cpool.tile([128, 8], FP32, tag="qkb", name="qkb_t")
            rows2_t = cpool.tile([1, 1032], BF16, tag="rows2", name="rows2_t")
            nc.scalar.dma_start(qkb_t[:], qkb[:])
            nc.scalar.dma_start(rows2_t[:], rows2[:])
            nc.sync.dma_start(xt_c[:], d_xt[:])
            nc.scalar.dma_start(wq_c[:], d_wq[:])
            nc.sync.dma_start(wv_c[:], d_wv[:])
            nc.scalar.dma_start(wk_c[:], d_wk[:])
            nc.sync.dma_start(mk_t[:], d_mk[:])
            nc.scalar.dma_start(ow_c[:], d_ow[:])

            xt_t = [xt_c[:, LK * k : LK * k + LK] for k in range(4)]
            wq_t = [wq_c[:, 512 * k : 512 * k + 512] for k in range(4)]
            wk_t = [wk_c[:, 512 * k : 512 * k + 512] for k in range(4)]
            wv_t = [wv_c[:, 520 * k : 520 * k + 520] for k in range(4)]
            ow_t = [ow_c[:, 512 * k : 512 * k + 512] for k in range(4)]
            vb_row = rows2_t[0:1, 0:520]
            ob_row = rows2_t[0:1, 520:1032]

            # HAM warmup: dummy matmuls with no DMA deps run while the input
            # DMAs stream in, so the PE clock gate is at 8/8 when the first
            # projection matmul issues. 'sc' tag slots have no user until QK0.
            for i in range(20):
                psd = ps.tile([128, 1536], FP32, tag="sc", name=f"dum{i}")
                nc.tensor.matmul(
                    psd[:, 0:512], dum[:, 0:128], dum[:, 0:512],
                    start=True, stop=True,
                )

            def keep_warm(tag, n):
                # no-dep dummy matmuls fill otherwise-idle PE slots so the
                # HAM activity monitor never re-throttles the clock to 4/8
                for i in range(n):
                    psd = ps.tile([128, 512], FP32, tag="ps1", name=f"kw_{tag}_{i}")
                    nc.tensor.matmul(
                        psd[:], dum[:, 0:128], dum[:, 0:512], start=True, stop=True,
                    )

            # ---- projections (emitted per pair, interleaved with QK) ----
            # qT[p]: [128 ch(2 heads), 512 q]; kT[p]: [128 ch, 768 keys]
            # biases applied on ACT during evacuation (per-partition bias AP)
            qT, kT = [None] * 4, [None] * 4

            def emit_qkproj(p):
                psq = ps.tile([128, 512], FP32, tag="ps1", name=f"psq{p}")
                for kk in range(4):
                    nc.tensor.matmul(
                        psq[:],
                        wq_t[kk][:, 128 * p : 128 * p + 128],
                        xt_t[kk][:, 128 : 128 + CH],
                        start=(kk == 0), stop=(kk == 3),
                    )
                q = ppool.tile([128, CH], BF16, tag=f"qT{p}", name=f"qT{p}")
                nc.scalar.activation(q[:], psq[:], Ident, bias=qkb_t[:, p : p + 1])
                qT[p] = q
                k_ = ppool.tile([128, LK], BF16, tag=f"kT{p}", name=f"kT{p}")
                kT[p] = k_
                for h, (c0, cw) in enumerate([(0, 512), (512, 256)]):
                    psk = ps.tile([128, cw], FP32, tag="ps1", name=f"psk{p}_{h}")
                    for kk in range(4):
                        nc.tensor.matmul(
                            psk[:],
                            wk_t[kk][:, 128 * p : 128 * p + 128],
                            xt_t[kk][:, c0 : c0 + cw],
                            start=(kk == 0), stop=(kk == 3),
                        )
                    nc.scalar.activation(
                        k_[:, c0 : c0 + cw], psk[:], Ident,
                        bias=qkb_t[:, 4 + p : 5 + p],
                    )

            # v in natural layout [keys, 8*(64+1)]: per head 64 v-cols + a
            # ones column (built by the K=1 bias-row matmul: wv_ext has zero
            # weight cols there and vb_row has 1.0) -> denominators free.
            v_t = []

            def emit_v(m):
                v = ppool.tile([128, 520], BF16, tag=f"v{m}", name=f"v{m}")
                for hh in range(2):
                    c0 = 260 * hh
                    psv = ps.tile([128, 260], FP32, tag="ps1", name=f"psv{m}_{hh}")
                    for kk in range(4):
                        nc.tensor.matmul(
                            psv[:],
                            xt_t[kk][:, 128 * m : 128 * m + 128],
                            wv_t[kk][:, c0 : c0 + 260],
                            start=(kk == 0), stop=False,
                        )
                    nc.tensor.matmul(
                        psv[:], ones1[:], vb_row[:, c0 : c0 + 260],
                        start=False, stop=True,
                    )
                    # split evacuations across ACT and DVE (both ~idle here)
                    if (m + hh) % 2 == 0:
                        nc.vector.tensor_copy(v[:, c0 : c0 + 260], psv[:])
                    else:
                        nc.scalar.activation(v[:, c0 : c0 + 260], psv[:], Copy)
                v_t.append(v)

            # ---- attention ----
            att_tiles = {}
            pso_tiles = {}
            vals_tiles = {}
            rbs_tiles = {}
            vT = [None] * 4

            def emit_qk(p):
                # row-packed pairs: head A on PE rows 0-63, head B on 64-127,
                # concurrent via tile_position auto-derive from base_partition
                scA = ps.tile([128, WSUM], FP32, tag="sc", name=f"scA{p}")
                scB = ps.tile([128, WSUM], FP32, tag="sc", name=f"scB{p}")
                att_tiles[p] = (
                    apool.tile([128, WSUM], BF16, tag="attA", name=f"attA{p}"),
                    apool.tile([128, WSUM], BF16, tag="attB", name=f"attB{p}"),
                )
                for (t, qs, w, off) in SEGS:
                    nc.tensor.matmul(
                        scA[:, off : off + w],
                        kT[p][0:64, 128 * t : 128 * t + 128],
                        qT[p][0:64, qs : qs + w],
                        start=True, stop=True,
                    )
                    nc.tensor.matmul(
                        scB[:, off : off + w],
                        kT[p][64:128, 128 * t : 128 * t + 128],
                        qT[p][64:128, qs : qs + w],
                        start=True, stop=True,
                    )
                return scA, scB

            def emit_exp_masks(p, scA, scB):
                attA, attB = att_tiles[p]
                nc.scalar.activation(attA[:], scA[:], Exp)
                nc.scalar.activation(attB[:], scB[:], Exp)
                nc.vector.tensor_tensor(attA[:], attA[:], mk_t[:], op=AluOpType.mult)
                nc.vector.tensor_tensor(attB[:], attB[:], mk_t[:], op=AluOpType.mult)

            def emit_av(p):
                attA, attB = att_tiles[p]
                psoA = ps.tile([HD + 1, CH], FP32, tag="ps1", name=f"psoA{p}")
                psoB = ps.tile([HD + 1, CH], FP32, tag="ps1", name=f"psoB{p}")
                pso_tiles[p] = (psoA, psoB)
                for hh, (pso, att) in enumerate([(psoA, attA), (psoB, attB)]):
                    head = 2 * p + hh
                    for t in range(NT):
                        w, qs, off = W_T[t], QS_T[t], OFF_T[t]
                        nc.tensor.matmul(
                            pso[:, qs : qs + w],
                            v_t[t][:, 65 * head : 65 * head + 65],
                            att[:, off : off + w],
                            start=(t == 0), stop=(t == NT - 1),
                            skip_group_check=True,
                        )
                # evacuate both psO tiles (64 value rows + den row) as bf16:
                # frees the psO banks immediately, so pair pipelining is
                # gated only by these fast copies, not the recip chain
                valsA = spool.tile([HD + 1, CH], BF16, tag="valsA", name=f"valsA{p}")
                valsB = spool.tile([HD + 1, CH], BF16, tag="valsB", name=f"valsB{p}")
                nc.vector.tensor_copy(valsA[:], psoA[:])
                nc.vector.tensor_copy(valsB[:], psoB[:])
                # DVE lanes cannot shift partitions; DMA-hop B's values to
                # partitions 64:128 so vtn_B is a lane-aligned two-SBUF op
                vB64 = spool.tile([128, CH], BF16, tag="vB64", name=f"vB64_{p}")
                nc.sync.dma_start(vB64[64:128, :], valsB[0:HD, :])
                vals_tiles[p] = (valsA, valsB, vB64)

            def emit_rbc(p):
                # broadcast den rows to 64 partitions each via K=1 bf16
                # matmuls, then 1/x = exp(-ln x) on ACT. The B-half recip
                # (partitions 64:128) is DMA-hopped to a base-0 tile so vtn
                # can be a two-SBUF-input op (equal-base-partition rule).
                valsA, valsB, _ = vals_tiles[p]
                rbc = ps.tile([128, CH], FP32, tag="sc", name=f"rbc{p}")
                nc.tensor.matmul(
                    rbc[0:64, :], ones64h[64:65, :], valsA[HD : HD + 1, :],
                    start=True, stop=True,
                )
                nc.tensor.matmul(
                    rbc[64:128, :], ones64h[64:65, :], valsB[HD : HD + 1, :],
                    start=True, stop=True, skip_group_check=True,
                )
                lnv = spool.tile([128, CH], FP32, tag="lnv", name=f"lnv{p}")
                nc.scalar.activation(lnv[:], rbc[:], Ln)
                rbs = spool.tile([128, CH], BF16, tag="rbs", name=f"rbs{p}")
                nc.scalar.activation(rbs[:], lnv[:], Exp, scale=-1.0)
                rbs_tiles[p] = rbs

            def emit_vtn(p):
                valsA, _, vB64 = vals_tiles[p]
                rbs = rbs_tiles[p]
                vtn = ppool.tile([128, CH], BF16, tag=f"vT{p}", name=f"vT{p}")
                nc.vector.tensor_tensor(
                    vtn[0:64, :], valsA[0:HD, :], rbs[0:64, :], op=AluOpType.mult
                )
                nc.vector.tensor_tensor(
                    vtn[64:128, :], vB64[64:128, :], rbs[64:128, :], op=AluOpType.mult
                )
                vT[p] = vtn

            # phase-1 schedule: QK pairs + exps + masks interleaved into
            # the projection stream so ACT's 4.5us/pair attention chain
            # overlaps PE-bound projection work instead of pacing phase 2
            emit_qkproj(0)
            sc0 = emit_qk(0)
            emit_exp_masks(0, *sc0)
            emit_qkproj(1)
            emit_qkproj(2)
            sc1 = emit_qk(1)
            emit_exp_masks(1, *sc1)
            emit_qkproj(3)
            sc2 = emit_qk(2)
            emit_exp_masks(2, *sc2)
            for m in range(4):
                emit_v(m)
            sc3 = emit_qk(3)
            emit_exp_masks(3, *sc3)
            emit_v(4)
            emit_v(5)
            # phase 2: AV + normalization chains trail lazily (psO banks are
            # freed by the bf16 CASTs, not the chains)
            emit_av(0)
            emit_av(1)
            emit_rbc(0)
            emit_vtn(0)
            emit_av(2)
            emit_rbc(1)
            emit_vtn(1)
            emit_av(3)
            emit_rbc(2)
            emit_vtn(2)
            emit_rbc(3)
            keep_warm("d", 3)

            # ---- output projection (ob via K=1 bias row) ----
            # two 3-bank psf tiles hold all four 512-col m-chunks, so the
            # p=0..2 partial sums run while pair 3's normalization chain
            # finishes (bridges the would-be PE gap; HAM stays at 8/8)
            psfX = ps.tile([128, WSUM], FP32, tag="sc", name="psfX")
            psfY = ps.tile([128, WSUM], FP32, tag="sc", name="psfY")
            psf_m = [
                psfX[:, 0:512], psfX[:, 512:1024], psfX[:, 1024:1536],
                psfY[:, 0:512],
            ]
            for p in range(3):
                for m in range(4):
                    nc.tensor.matmul(
                        psf_m[m],
                        vT[p][:, 128 * m : 128 * m + 128],
                        ow_t[p][:],
                        start=(p == 0), stop=False, skip_group_check=True,
                    )
            emit_vtn(3)
            for m in range(4):
                nc.tensor.matmul(
                    psf_m[m],
                    vT[3][:, 128 * m : 128 * m + 128],
                    ow_t[3][:],
                    start=False, stop=False, skip_group_check=True,
                )
                nc.tensor.matmul(
                    psf_m[m], ones1[:], ob_row[:], start=False, stop=True,
                    skip_group_check=True,
                )
                fin = spool.tile([128, E], FP32, tag="fin", bufs=4, name=f"fin{m}")
                if m % 2 == 0:
                    nc.scalar.activation(fin[:], psf_m[m], Copy)
                    nc.sync.dma_start(out[128 * m : 128 * m + 128, :], fin[:])
                else:
                    nc.vector.tensor_copy(fin[:], psf_m[m])
                    nc.scalar.dma_start(out[128 * m : 128 * m + 128, :], fin[:])

    return nc


_NC_CACHE = None


def _get_program():
    global _NC_CACHE
    if _NC_CACHE is None:
        _NC_CACHE = _build_program()
    return _NC_CACHE


def _make_in_maps(x, padding_mask, qkv_w, qkv_b, o_w, o_b):
    x = np.asarray(x, np.float32)
    pm = np.asarray(padding_mask)
    qkv_w = np.asarray(qkv_w, np.float32)
    qkv_b = np.asarray(qkv_b, np.float32)
    o_w = np.asarray(o_w, np.float32)
    o_b = np.asarray(o_b, np.float32)

    scale = np.float32(1.0 / np.sqrt(HD))
    # reference splits per-head: head h uses qkv rows [192h,192h+64) (q),
    # +64 (k), +128 (v)
    idx_q = np.concatenate([np.arange(3 * HD * h, 3 * HD * h + HD) for h in range(H)])
    idx_k = idx_q + HD
    idx_v = idx_q + 2 * HD

    wq = (qkv_w[idx_q] * scale).T.astype(BFNP)        # [IN, E]
    wk = qkv_w[idx_k].T.astype(BFNP)
    wv = qkv_w[idx_v].T.astype(BFNP)                  # [IN, E]
    # wv_ext: per head 64 cols + zero col (ones come from the bias row)
    wv_ext = np.zeros((IN_DIM, 520), BFNP)
    vb_ext = np.zeros((520,), BFNP)
    for h in range(H):
        wv_ext[:, 65 * h : 65 * h + 64] = wv[:, 64 * h : 64 * h + 64]
        vb_ext[65 * h : 65 * h + 64] = qkv_b[idx_v][64 * h : 64 * h + 64].astype(BFNP)
        vb_ext[65 * h + 64] = BFNP(1.0)
    ow = o_w.T.astype(BFNP)                           # [E_in, E_out]

    qkb = np.zeros((128, 8), np.float32)
    qkb[:, 0:4] = (qkv_b[idx_q] * scale).reshape(4, 128).T
    qkb[:, 4:8] = qkv_b[idx_k].reshape(4, 128).T

    rows2 = np.zeros((1, 1032), BFNP)
    rows2[0, 0:520] = vb_ext
    rows2[0, 520:1032] = o_b.astype(BFNP)

    # weight concats (identical for all cores): [128, 4*cols]
    cat = lambda w, c: np.ascontiguousarray(
        np.concatenate([w[128 * j : 128 * j + 128, :] for j in range(4)], axis=1)
    )
    d_wq, d_wk, d_ow = cat(wq, 512), cat(wk, 512), cat(ow, 512)
    d_wv = cat(wv_ext, 520)

    j = np.arange(128)[:, None]
    in_maps = []
    for c in range(8):
        b, ch = divmod(c, 4)
        s0 = CH * ch
        lo, hi = max(0, s0 - HW), min(S, s0 + CH + HW)
        xpad = np.zeros((LK, IN_DIM), np.float32)
        xpad[lo - (s0 - HW) : hi - (s0 - HW)] = x[b, lo:hi]
        xt = np.ascontiguousarray(xpad.T).astype(BFNP)        # [IN, LK]
        d_xt = np.ascontiguousarray(
            np.concatenate([xt[128 * k : 128 * k + 128, :] for k in range(4)], axis=1)
        )

        mask = np.zeros((128, WSUM), np.float32)
        for t in range(NT):
            w, qs, off = W_T[t], QS_T[t], OFF_T[t]
            lk = 128 * t + j                                  # [128,1] local key
            q = qs + np.arange(w)[None, :]                    # [1,w] local query
            band = (q <= lk) & (lk <= q + 2 * HW)
            gk = s0 - HW + lk                                 # global key index
            valid = (gk >= 0) & (gk < S)
            pmk = pm[b, np.clip(gk, 0, S - 1)] != 0
            mask[:, off : off + w] = (band & valid & pmk).astype(np.float32)

        in_maps.append(
            {
                "d_xt": d_xt,
                "d_wq": d_wq,
                "d_wk": d_wk,
                "d_wv": d_wv,
                "d_ow": d_ow,
                "d_mk": np.ascontiguousarray(mask.astype(BFNP)),
                "qkb": qkb,
                "rows2": rows2,
            }
        )
    return in_maps


def _run(x, padding_mask, qkv_w, qkv_b, o_w, o_b, trace=False, tmpdir=None):
    nc = _get_program()
    in_maps = _make_in_maps(x, padding_mask, qkv_w, qkv_b, o_w, o_b)
    res = run_bass_kernel_spmd(
        nc, in_maps, core_ids=list(range(8)), trace=trace, tmpdir=tmpdir
    )
    o = np.empty((B, S, E), np.float32)
    for c in range(8):
        b, ch = divmod(c, 4)
        o[b, CH * ch : CH * ch + CH, :] = res.results[c]["out"]
    # fully-masked query rows: att = 0 -> output is exactly the bias
    pm = np.asarray(padding_mask)
    if (pm == 0).any():
        o[pm == 0] = np.asarray(o_b, np.float32)
    return o, res


def kernel(x, padding_mask, qkv_w, qkv_b, o_w, o_b, window_size, num_heads):
    assert int(window_size) == WS and int(num_heads) == H
    assert tuple(np.asarray(x).shape) == (B, S, IN_DIM)
    o, _ = _run(x, padding_mask, qkv_w, qkv_b, o_w, o_b)
    return o
